# revision 1
# baseline (speedup 1.0000x reference)
"""Ewald reciprocal-space kernel for Trainium2 (8 NeuronCores, SPMD).

Math (per batch b):
    s        = cell_inv @ x          (fractional coords)
    theta    = 2*pi * (kvec . s)     (B, N, NK) phases
    S_re/S_im= sum_n q_n {cos,sin}(theta)          (structure factor)
    recip_n  = sum_k expfac_k (S_re cos + S_im sin)
    phi      = recip * BOHR/(pi*V) - q * 2*bewald*BOHR/sqrt(pi)
    returns (0.5*q*phi, phi)

Sharding: 8 cores = 2 batches x 4 k-shards (1024 k-vectors each). Each core
computes its full-N, shard-K contribution to recip with no collectives; host
sums the 4 shard partials per batch and applies the final affine.

Device pipeline per core (N=4096 as 32 chunks of 128 partitions):
  u = x . (Cinv^T k)  [= theta/2pi]   via fp32r matmul (contraction dim 3)
  rn = (u + M) - M            magic-number round-to-nearest (DVE tensor_scalar)
  -r = rn - u                 in [-1/2, 1/2]   (DVE scalar_tensor_tensor)
  -r_c = wrap(-r - 1/4)       in [-1/2, 1/2]   (DVE add_range_wrap custom op)
  sin(theta) = Sin(-2pi * -r), cos(theta) = Sin(-2pi * -r_c)  (ACT, fp16 out)
  S_re/S_im: PE matmuls contracting n with q as weights (psum accumulate)
  cs chunks DMA-transposed (xbar) into csT[k-slice partitions, n free]
  w = expfac * S  (small), transposed to a [128,16] column tile via DRAM bounce
  recip: PE matmuls contracting k-slices: sum_j wcol_j^T @ csT_j
"""

import math
from contextlib import ExitStack

import numpy as np

BOHR = 1.8897261258369282

B, N, NK = 2, 4096, 4096
NCORES = 8
KSH = NK // 4          # k-vectors per core
NCH = N // 128         # 32 n-chunks
CW = 2 * KSH           # cs chunk width: [cos | sin]
NSL = CW // 128        # 16 k-slices per chunk

_PROG = {}


def _build_program(reps: int = 1, stage: str = 'full'):
    import concourse.bass as bass
    import concourse.bacc as bacc
    import concourse.tile as tile
    import concourse.mybir as mybir

    F32 = mybir.dt.float32
    F32R = mybir.dt.float32r
    F16 = mybir.dt.float16
    MAGIC = 12582912.0          # 1.5 * 2**23: fp32 round-to-nearest-integer
    # two fp32 ulps below 2*pi so |scale * r| <= pi holds for r = +-1/2 exactly
    NEG2PI = -6.28318452835083
    ADD = mybir.AluOpType.add
    SUB = mybir.AluOpType.subtract

    nc = bacc.Bacc(trn_type="TRN2", target_bir_lowering=False, debug=False)

    coordsT_d = nc.dram_tensor("coordsT", [3, N], F32, kind="ExternalInput").ap()
    qT_d = nc.dram_tensor("qT", [128, NCH], F32, kind="ExternalInput").ap()
    cinv_d = nc.dram_tensor("cinv", [3, 3], F32, kind="ExternalInput").ap()
    kvecT_d = nc.dram_tensor("kvecT", [3, KSH], F32, kind="ExternalInput").ap()
    expfac_d = nc.dram_tensor("expfac", [1, KSH], F32, kind="ExternalInput").ap()
    recip_d = nc.dram_tensor("recip", [1, N], F32, kind="ExternalOutput").ap()
    wb_d = nc.dram_tensor("w_bounce", [1, CW], F16)

    rep_markers = []
    with tile.TileContext(nc) as tc, ExitStack() as ctx:
        const = ctx.enter_context(tc.tile_pool(name="const", bufs=1))
        pu = ctx.enter_context(tc.tile_pool(name="pu", bufs=2, space="PSUM"))
        pacc = ctx.enter_context(tc.tile_pool(name="pacc", bufs=1, space="PSUM"))
        wk_rn = ctx.enter_context(tc.tile_pool(name="wk_rn", bufs=2))
        wk_mr = ctx.enter_context(tc.tile_pool(name="wk_mr", bufs=3))
        wk_cs = ctx.enter_context(tc.tile_pool(name="wk_cs", bufs=4))
        wk_out = ctx.enter_context(tc.tile_pool(name="wk_out", bufs=2))

        # ---- load inputs ----
        kvt = wk_cs.tile([3, KSH], F32R, tag="cs", name="kvt")
        nc.sync.dma_start(out=kvt[:, :], in_=kvecT_d.bitcast(F32R))
        cinv_t = const.tile([3, 3], F32R)
        nc.sync.dma_start(out=cinv_t[:, :], in_=cinv_d.bitcast(F32R))
        cts = const.tile([3, N], F32R)
        nc.sync.dma_start(out=cts[:, 0:1024], in_=coordsT_d[:, 0:1024].bitcast(F32R))
        qt = const.tile([128, NCH], F32)
        nc.sync.dma_start(out=qt[:, :], in_=qT_d)
        for h in range(1024, N, 1024):
            nc.sync.dma_start(
                out=cts[:, h : h + 1024],
                in_=coordsT_d[:, h : h + 1024].bitcast(F32R),
            )
        ef_a = wk_out.tile([1, 512], F32, tag="rr", name="ef_a")
        nc.sync.dma_start(out=ef_a[:, :], in_=expfac_d[:, 0:512])
        ef_b = wk_out.tile([1, 512], F32, tag="rr", name="ef_b")
        nc.sync.dma_start(out=ef_b[:, :], in_=expfac_d[:, 512:1024])

        qt16 = const.tile([128, NCH], F16)
        nc.scalar.copy(qt16[:, :], qt[:, :])

        # persistent across reps: the transposed trig store
        csT = const.tile([128, NSL, N], F16)   # [k-in-slice][slice j][n]
        for _rep in range(reps):
            if _rep > 0:
                tc.strict_bb_all_engine_barrier()
            # ---- kmodT[j, k] = sum_i cinv[i, j] * kvecT[i, k]  (= (Cinv^T k)^T) ----
            km_ps = pu.tile([128, KSH], F32, tag="u")
            for h in range(0, KSH, 512):
                nc.tensor.matmul(
                    km_ps[:3, h : h + 512], lhsT=cinv_t[:, :], rhs=kvt[:, h : h + 512],
                    start=True, stop=True,
                )
            kmod = const.tile([3, KSH], F32R)
            nc.vector.tensor_copy(kmod[:, 0:512], km_ps[:3, 0:512])
            nc.vector.tensor_copy(kmod[:, 512:1024], km_ps[:3, 512:1024])

            # chunks whose round-to-nearest runs on the scalar engine (balances
            # DVE vs ACT busy time; ~19/32 assisted)
            ASSIST = {int((i + 0.5) * NCH / 19) for i in range(19)}

            sab = [
                pacc.tile([1, 512], F32, tag=f"sab{j}", name=f"sab{j}") for j in range(4)
            ]

            # ---- pass 1: phases, trig, structure factors, transposes ----
            # Software-pipelined one chunk ahead: the (matmul -> round-to-nearest)
            # production for chunk t+1 is emitted before chunk t's Sin
            # activations, so the DVE's scalar_tensor_tensor never waits on a
            # busy scalar engine.
            def produce(t):
                u_ps = pu.tile([128, KSH], F32, tag="u", name=f"u{t}")
                for h in range(0, KSH, 512):
                    nc.tensor.matmul(
                        u_ps[:, h : h + 512],
                        lhsT=cts[:, 128 * t : 128 * (t + 1)],
                        rhs=kmod[:, h : h + 512],
                        start=True, stop=True,
                    )
                rn = wk_rn.tile([128, KSH], F32, tag="rn", name=f"rn{t}")
                if t in ASSIST and stage not in ('mm', 'rr'):
                    # scalar engine computes v = u + M; DVE then gets rn - u via
                    # (v - M) - u in one scalar_tensor_tensor
                    nc.scalar.activation(
                        rn[:, :], u_ps[:, :],
                        mybir.ActivationFunctionType.Copy, bias=MAGIC, scale=1.0,
                    )
                    s0 = MAGIC
                else:
                    nc.vector.tensor_scalar(
                        out=rn[:, :], in0=u_ps[:, :], scalar1=MAGIC, scalar2=MAGIC,
                        op0=ADD, op1=SUB,
                    )
                    s0 = 0.0
                return u_ps, rn, s0

            cur = produce(0)
            for t in range(NCH):
                u_ps, rn, s0 = cur
                if stage == 'mm':
                    dummy = wk_rn.tile([128, KSH], F32, tag="rn", name=f"d{t}")
                    nc.vector.tensor_copy(dummy[:, :], u_ps[:, :])
                    if t + 1 < NCH:
                        cur = produce(t + 1)
                    continue
                mm = wk_mr.tile([128, CW], F32)    # [-r | -r_c] halves
                nc.vector.scalar_tensor_tensor(
                    out=mm[:, 0:KSH], in0=rn[:, :], scalar=s0, in1=u_ps[:, :],
                    op0=ADD if s0 == 0.0 else SUB, op1=SUB,
                )
                nc.vector.add_range_wrap(
                    out=mm[:, KSH:CW], in_=mm[:, 0:KSH],
                    shift=-0.25, bound=0.5, period=1.0,
                )
                if t + 1 < NCH:
                    cur = produce(t + 1)
                if stage == 'rr':
                    continue
                # one Sin over both halves: cs = [sin(theta) | cos(theta)]
                cs = wk_cs.tile([128, CW], F16, tag="cs")
                nc.scalar.activation(
                    cs[:, :], mm[:, :],
                    mybir.ActivationFunctionType.Sin, bias=0.0, scale=NEG2PI,
                )
                for j in range(4):
                    nc.tensor.matmul(
                        sab[j][:, :],
                        lhsT=qt16[:, t : t + 1],
                        rhs=cs[:, 512 * j : 512 * (j + 1)],
                        start=(t == 0), stop=(t == NCH - 1),
                    )
                if stage == 'act':
                    continue
                # csT[p, j, 128t + n] = cs[n, 128j + p]
                nc.sync.dma_start_transpose(
                    out=csT[:, :, 128 * t : 128 * (t + 1)], in_=cs[:, :],
                )

            if stage != 'full':
                zz = wk_out.tile([1, 512], F32, tag="rr", name="zz")
                nc.vector.memset(zz[:, :], 0.0)
                for nf in range(0, N, 512):
                    nc.sync.dma_start(out=recip_d[:, nf : nf + 512], in_=zz[:, :])
                continue
            # ---- mid: w = expfac * S; transpose to column layout via DRAM ----
            w_row = const.tile([1, CW], F16, tag="w_row")
            for j in range(4):
                nc.vector.tensor_tensor(
                    out=w_row[:, 512 * j : 512 * (j + 1)],
                    in0=sab[j][:, :],
                    in1=(ef_a if j % 2 == 0 else ef_b)[:, :],
                    op=mybir.AluOpType.mult,
                )
            nc.sync.dma_start(out=wb_d.ap(), in_=w_row[:, :])
            wcolT = const.tile([128, NSL], F16, tag="wcolT")
            nc.sync.dma_start_transpose(
                out=wcolT[:, :], in_=wb_d.ap().rearrange("a (j p) -> (a j) p", p=128),
            )

            # ---- pass 2: recip = sum_j wcol_j^T @ csT_j  (contract k on PE) ----
            for nf in range(0, N, 512):
                pb = pu.tile([1, 512], F32, tag="u", name="pb")
                for j in range(NSL):
                    nc.tensor.matmul(
                        pb[:, :],
                        lhsT=wcolT[:, j : j + 1],
                        rhs=csT[:, j, nf : nf + 512],
                        start=(j == 0), stop=(j == NSL - 1),
                    )
                rr = wk_out.tile([1, 512], F32)
                nc.scalar.copy(rr[:, :], pb[:, :])
                nc.sync.dma_start(out=recip_d[:, nf : nf + 512], in_=rr[:, :])

    nc.compile()
    return nc


def _get_prog(reps: int = 1, stage: str = "full"):
    key = (reps, stage)
    if key not in _PROG:
        _PROG[key] = _build_program(reps, stage)
    return _PROG[key]


def _make_in_maps(coords, q, cell_inv, kvec, expfac):
    in_maps = []
    for c in range(NCORES):
        b, ks = divmod(c, NCORES // B)
        sl = slice(KSH * ks, KSH * (ks + 1))
        in_maps.append({
            "coordsT": np.ascontiguousarray(coords[b].T, dtype=np.float32),
            "qT": np.ascontiguousarray(q[b].reshape(NCH, 128).T, dtype=np.float32),
            "cinv": np.ascontiguousarray(cell_inv, dtype=np.float32),
            "kvecT": np.ascontiguousarray(kvec[sl].T, dtype=np.float32),
            "expfac": np.ascontiguousarray(expfac[sl][None, :], dtype=np.float32),
        })
    return in_maps


def _finalize(results, q, volume, bewald):
    recip = np.zeros((B, N), np.float32)
    for c in range(NCORES):
        b = c // (NCORES // B)
        recip[b] += results[c]["recip"][0]
    scale1 = np.float32(BOHR / (math.pi * float(volume[0])))
    scale2 = np.float32(2.0 * float(bewald[0]) * BOHR / math.sqrt(math.pi))
    phi = (recip * scale1 - q.astype(np.float32) * scale2).astype(np.float32)
    e = (np.float32(0.5) * q.astype(np.float32) * phi).astype(np.float32)
    return e, phi


def kernel(coords, q, cell_inv, kvec, expfac, volume, bewald):
    from concourse.bass_utils import run_bass_kernel_spmd

    nc = _get_prog()
    in_maps = _make_in_maps(coords, q, cell_inv, kvec, expfac)
    res = run_bass_kernel_spmd(nc, in_maps, list(range(NCORES))).results
    return _finalize(res, q, volume, bewald)



# revision 2
# speedup vs baseline: 1.4090x; 1.4090x over previous
"""Ewald reciprocal-space kernel for Trainium2 (8 NeuronCores, SPMD) — v3.

Math (per batch b):
    s        = cell_inv @ x          (fractional coords)
    theta    = 2*pi * (kvec . s)     (B, N, NK) phases
    S_re/S_im= sum_n q_n {cos,sin}(theta)          (structure factor)
    recip_n  = sum_k expfac_k (S_re cos + S_im sin)
    phi      = recip * BOHR/(pi*V) - q * 2*bewald*BOHR/sqrt(pi)
    returns (0.5*q*phi, phi)

Sharding: 8 cores = 2 batches x 4 k-shards (1024 k-vectors each). Each core
computes its full-N, shard-K contribution to recip with no collectives; host
sums the 4 shard partials per batch and applies the final affine.

Device pipeline per core (N=4096 as 32 chunks of 128 partitions):
  u  = x . kmod (kmod = Cinv^T k)       fp32r matmul into PSUM
  rn = (u + M) - M                      magic round; DVE or ACT-assisted
  f  = rn - u  in [-1/2, 1/2]           DVE scalar_tensor_tensor, fp16 out
  g  = |f|     (uint16 AND 0x7FFF)      DVE 4x-mode fp16
  h  = g - 1/4                          GPSIMD (pool), fp16
  sin(theta) = Sin(-2pi f), cos(theta) = Sin(-2pi h)   one ACT Sin per 2 chunks
  S rows: 4 PE matmuls/chunk, out [1, 512] parked at quadrant partition 32q
     (4 concurrent PSUM groups share one 2KB region on disjoint partitions)
  cs chunks DMA-transposed (xbar) into csT[k-slice partitions, n free]
  S extraction: 4 single-partition row copies (DVE+ACT), then 16 tiny PE
     transposes [1,128]->[128,1] into one PSUM group; w = efT * S^T (DVE)
  recip: free-size-1 weight-stationary PE matmuls (lhsT = csT block,
     rhs = w column), accumulated over 16 k-slices; PSUM ring reuse with
     per-pair copy-out, recip emitted as [128, 32]
"""

import math
from contextlib import ExitStack

import numpy as np

BOHR = 1.8897261258369282

B, N, NK = 2, 4096, 4096
NCORES = 8
KSH = NK // 4          # k-vectors per core
NCH = N // 128         # 32 n-chunks
CW = 2 * KSH           # cs chunk width: [sin | cos]
NSL = CW // 128        # 16 k-slices per chunk
MAGIC = 12582912.0     # 1.5 * 2**23: fp32 round-to-nearest-integer
NEG2PI = -6.28318452835083  # two fp32 ulps below 2*pi

_PROG = {}


def _build_program(reps: int = 1, stage: str = 'full', n_assist: int = 10,
                   mm_bufs: int = 2, rn_bufs: int = 2, cs_bufs: int = 3):
    import concourse.bass as bass
    import concourse.bacc as bacc
    import concourse.tile as tile
    import concourse.mybir as mybir

    F32 = mybir.dt.float32
    F32R = mybir.dt.float32r
    F16 = mybir.dt.float16
    U16 = mybir.dt.uint16
    ADD = mybir.AluOpType.add
    SUB = mybir.AluOpType.subtract
    MULT = mybir.AluOpType.mult
    AND = mybir.AluOpType.bitwise_and
    SIN = mybir.ActivationFunctionType.Sin
    COPY = mybir.ActivationFunctionType.Copy

    nc = bacc.Bacc(trn_type="TRN2", target_bir_lowering=False, debug=False)

    coordsT_d = nc.dram_tensor("coordsT", [3, N], F32, kind="ExternalInput").ap()
    qT_d = nc.dram_tensor("qT", [128, NCH], F32, kind="ExternalInput").ap()
    kmodT_d = nc.dram_tensor("kmodT", [3, KSH], F32, kind="ExternalInput").ap()
    efT_d = nc.dram_tensor("efT", [128, NSL], F32, kind="ExternalInput").ap()
    recip_d = nc.dram_tensor("recip", [128, NCH], F32, kind="ExternalOutput").ap()

    # chunks whose rounding runs on the scalar engine (DVE/ACT balance)
    ASSIST = {int((i + 0.5) * NCH / n_assist) for i in range(n_assist)}

    with tile.TileContext(nc) as tc, ExitStack() as ctx:
        const = ctx.enter_context(tc.tile_pool(name="const", bufs=1))
        pu = ctx.enter_context(tc.tile_pool(name="pu", bufs=3, space="PSUM"))
        pacc = ctx.enter_context(tc.tile_pool(name="pacc", bufs=1, space="PSUM"))
        wk_rn = ctx.enter_context(tc.tile_pool(name="wk_rn", bufs=rn_bufs))
        wk_mm = ctx.enter_context(tc.tile_pool(name="wk_mm", bufs=mm_bufs))
        wk_cs = ctx.enter_context(tc.tile_pool(name="wk_cs", bufs=cs_bufs))
        wk_out = ctx.enter_context(tc.tile_pool(name="wk_out", bufs=1))

        # ---- load inputs ----
        kmod = const.tile([3, KSH], F32R, name="kmod")
        nc.sync.dma_start(out=kmod[:, :], in_=kmodT_d.bitcast(F32R))
        cts = const.tile([3, N], F32R)
        for hh in range(0, N, 1024):
            nc.sync.dma_start(
                out=cts[:, hh:hh + 1024],
                in_=coordsT_d[:, hh:hh + 1024].bitcast(F32R),
            )
        qt = const.tile([128, NCH], F32)
        nc.sync.dma_start(out=qt[:, :], in_=qT_d)
        efT = const.tile([128, NSL], F32)
        nc.sync.dma_start(out=efT[:, :], in_=efT_d)

        qt16 = const.tile([128, NCH], F16)
        nc.vector.tensor_copy(qt16[:, :], qt[:, :])
        one1 = const.tile([1, 1], F32)
        nc.vector.memset(one1[:, :], 1.0)

        # persistent: the transposed trig store and the S quadrant rows
        csT = const.tile([128, NSL, N], F16)   # [k-in-slice][slice j][n]
        sgrid = pacc.tile([128, 512], F32, tag="sg")

        def emit_u(t):
            u_ps = pu.tile([128, KSH], F32, tag="u", name=f"u{t}")
            for hh in range(0, KSH, 512):
                nc.tensor.matmul(
                    u_ps[:, hh:hh + 512],
                    lhsT=cts[:, 128 * t:128 * (t + 1)],
                    rhs=kmod[:, hh:hh + 512],
                    start=True, stop=True,
                )
            return u_ps

        us = {t: emit_u(t) for t in range(3)}
        for s in range(NCH // 2):
            mmp = wk_mm.tile([128, 2 * CW], F16, tag="mm", name=f"mm{s}")
            for c in (0, 1):
                t = 2 * s + c
                u_ps = us.pop(t)
                fsl = mmp[:, CW * c:CW * c + KSH]            # f (sin half)
                hsl = mmp[:, CW * c + KSH:CW * (c + 1)]      # h (cos half)
                rn = wk_rn.tile([128, KSH], F32, tag="rn", name=f"rn{t}")
                if t in ASSIST and stage != 'rr0':
                    # ACT: v = u + M (rounds); DVE stt: (v - M) - u
                    nc.scalar.activation(rn[:, :], u_ps[:, :], COPY,
                                         bias=MAGIC, scale=1.0)
                    s0 = MAGIC
                else:
                    nc.vector.tensor_scalar(
                        out=rn[:, :], in0=u_ps[:, :],
                        scalar1=MAGIC, scalar2=MAGIC, op0=ADD, op1=SUB,
                    )
                    s0 = 0.0
                nc.vector.scalar_tensor_tensor(
                    out=fsl, in0=rn[:, :], scalar=s0, in1=u_ps[:, :],
                    op0=ADD if s0 == 0.0 else SUB, op1=SUB,
                )
                # g = |f| via uint16 AND; h = g - 1/4 on the pool engine
                nc.vector.tensor_scalar(
                    out=hsl.bitcast(U16), in0=fsl.bitcast(U16),
                    scalar1=0x7FFF, scalar2=None, op0=AND,
                )
                nc.gpsimd.tensor_scalar(
                    out=hsl, in0=hsl, scalar1=0.25, scalar2=None, op0=SUB,
                )
                tn = t + 3
                if tn < NCH:
                    us[tn] = emit_u(tn)
            if stage == 'rr':
                continue
            # sin(theta) = Sin(-2pi f); cos(theta) = Sin(-2pi h)
            csp = wk_cs.tile([128, 2 * CW], F16, tag="cs", name=f"cs{s}")
            nc.scalar.activation(csp[:, :], mmp[:, :], SIN,
                                 bias=0.0, scale=NEG2PI)
            for c in (0, 1):
                t = 2 * s + c
                for qd in range(4):
                    nc.tensor.matmul(
                        sgrid[32 * qd:32 * qd + 1, 0:512],
                        lhsT=qt16[:, t:t + 1],
                        rhs=csp[:, CW * c + 512 * qd:CW * c + 512 * (qd + 1)],
                        start=(t == 0), stop=(t == NCH - 1),
                        tile_position=(0, 32 * qd),
                    )
                if stage != 'act':
                    # csT[p, j, 128t + n] = csp[n, CW*c + 128j + p]
                    nc.sync.dma_start_transpose(
                        out=csT[:, :, 128 * t:128 * (t + 1)],
                        in_=csp[:, CW * c:CW * (c + 1)],
                    )

        if stage != 'full':
            zz = wk_out.tile([128, NCH], F32, name="zz")
            nc.vector.memset(zz[:, :], 0.0)
            nc.sync.dma_start(out=recip_d, in_=zz[:, :])
        else:
            # ---- S extraction: 4 quadrant rows -> SBUF (DVE + ACT) ----
            sgq0 = const.tile([1, 512], F32, name="sgq0")
            sgq1 = const.tile([1, 512], F32, name="sgq1")
            sgq2 = const.tile([1, 512], F32, name="sgq2")
            sgq3 = const.tile([1, 512], F32, name="sgq3")
            sgq = [sgq0, sgq1, sgq2, sgq3]
            nc.vector.tensor_copy(sgq0[:, :], sgrid[0:1, 0:512])
            nc.scalar.copy(sgq1[:, :], sgrid[32:33, 0:512])
            nc.vector.tensor_copy(sgq2[:, :], sgrid[64:65, 0:512])
            nc.scalar.copy(sgq3[:, :], sgrid[96:97, 0:512])

            # ---- 16 PE transposes [1,128] -> [128,1], one psum group ----
            wtp = pacc.tile([128, 512], F32, tag="wtp")
            for j in range(NSL):
                qd, aa = j // 4, j % 4
                nc.tensor.matmul(
                    wtp[:, j:j + 1],
                    lhsT=sgq[qd][0:1, 128 * aa:128 * (aa + 1)],
                    rhs=one1[:, :],
                    is_transpose=True, start=(j == 0), stop=(j == NSL - 1),
                )
            wcolT = const.tile([128, NSL], F16, tag="wcolT")
            nc.vector.tensor_tensor(
                out=wcolT[:, :], in0=wtp[:, 0:NSL], in1=efT[:, :], op=MULT,
            )

            # ---- pass 2: recip[n] = sum_j csT[:, j, n]^T wcolT[:, j] ----
            rr = wk_out.tile([128, NCH], F32, name="rr")
            for m in range(NCH // 2):
                rp = pu.tile([128, KSH], F32, tag="u", name=f"rp{m}")
                for c in (0, 1):
                    t = 2 * m + c
                    for j in range(NSL):
                        nc.tensor.matmul(
                            rp[:, 512 * c:512 * c + 1],
                            lhsT=csT[:, j, 128 * t:128 * (t + 1)],
                            rhs=wcolT[:, j:j + 1],
                            start=(j == 0), stop=(j == NSL - 1),
                        )
                if m % 2 == 0:
                    nc.vector.tensor_copy(rr[:, 2 * m:2 * m + 2],
                                          rp[:, 0:KSH:512])
                else:
                    nc.scalar.copy(rr[:, 2 * m:2 * m + 2], rp[:, 0:KSH:512])
                if m % 4 == 3:
                    nc.sync.dma_start(
                        out=recip_d[:, 2 * m - 6:2 * m + 2],
                        in_=rr[:, 2 * m - 6:2 * m + 2],
                    )

    nc.compile()
    return nc


def _get_prog(reps: int = 1, stage: str = "full", **kw):
    key = (reps, stage, tuple(sorted(kw.items())))
    if key not in _PROG:
        _PROG[key] = _build_program(reps, stage, **kw)
    return _PROG[key]


def _make_in_maps(coords, q, cell_inv, kvec, expfac):
    in_maps = []
    for c in range(NCORES):
        b, ks = divmod(c, NCORES // B)
        sl = slice(KSH * ks, KSH * (ks + 1))
        # efT[p, j] = expfac_shard[(128j + p) mod 1024]
        ef = np.asarray(expfac[sl], dtype=np.float32).reshape(8, 128).T  # [128, 8]
        efT = np.concatenate([ef, ef], axis=1)                           # [128, 16]
        in_maps.append({
            "coordsT": np.ascontiguousarray(coords[b].T, dtype=np.float32),
            "qT": np.ascontiguousarray(q[b].reshape(NCH, 128).T, dtype=np.float32),
            "kmodT": np.ascontiguousarray(
                cell_inv.astype(np.float32).T @ kvec[sl].T.astype(np.float32)),
            "efT": np.ascontiguousarray(efT),
        })
    return in_maps


def _finalize(results, q, volume, bewald):
    recip = np.zeros((B, N), np.float32)
    for c in range(NCORES):
        b = c // (NCORES // B)
        recip[b] += results[c]["recip"].T.reshape(-1)
    scale1 = np.float32(BOHR / (math.pi * float(volume[0])))
    scale2 = np.float32(2.0 * float(bewald[0]) * BOHR / math.sqrt(math.pi))
    phi = (recip * scale1 - q.astype(np.float32) * scale2).astype(np.float32)
    e = (np.float32(0.5) * q.astype(np.float32) * phi).astype(np.float32)
    return e, phi


def kernel(coords, q, cell_inv, kvec, expfac, volume, bewald):
    from concourse.bass_utils import run_bass_kernel_spmd

    nc = _get_prog()
    in_maps = _make_in_maps(coords, q, cell_inv, kvec, expfac)
    res = run_bass_kernel_spmd(nc, in_maps, list(range(NCORES))).results
    return _finalize(res, q, volume, bewald)


# revision 3
# speedup vs baseline: 1.5210x; 1.0795x over previous
"""Ewald reciprocal-space kernel for Trainium2 (8 NeuronCores, SPMD) — v3.

Math (per batch b):
    s        = cell_inv @ x          (fractional coords)
    theta    = 2*pi * (kvec . s)     (B, N, NK) phases
    S_re/S_im= sum_n q_n {cos,sin}(theta)          (structure factor)
    recip_n  = sum_k expfac_k (S_re cos + S_im sin)
    phi      = recip * BOHR/(pi*V) - q * 2*bewald*BOHR/sqrt(pi)
    returns (0.5*q*phi, phi)

Sharding: 8 cores = 2 batches x 4 k-shards (1024 k-vectors each). Each core
computes its full-N, shard-K contribution to recip with no collectives; host
sums the 4 shard partials per batch and applies the final affine.

Device pipeline per core (N=4096 as 32 chunks of 128 partitions):
  u  = x . kmod (kmod = Cinv^T k)       fp32r matmul into PSUM
  rn = (u + M) - M                      magic round; DVE or ACT-assisted
  f  = rn - u  in [-1/2, 1/2]           DVE scalar_tensor_tensor, fp16 out
  g  = |f|     (uint16 AND 0x7FFF)      DVE 4x-mode fp16
  h  = g - 1/4                          GPSIMD (pool), fp16
  sin(theta) = Sin(-2pi f), cos(theta) = Sin(-2pi h)   one ACT Sin per 2 chunks
  S rows: 4 PE matmuls/chunk, out [1, 512] parked at quadrant partition 32q
     (4 concurrent PSUM groups share one 2KB region on disjoint partitions)
  cs chunks DMA-transposed (xbar) into csT[k-slice partitions, n free]
  S extraction: 4 single-partition row copies (DVE+ACT), then 16 tiny PE
     transposes [1,128]->[128,1] into one PSUM group; w = efT * S^T (DVE)
  recip: free-size-1 weight-stationary PE matmuls (lhsT = csT block,
     rhs = w column), accumulated over 16 k-slices; PSUM ring reuse with
     per-pair copy-out, recip emitted as [128, 32]
"""

import math
from contextlib import ExitStack

import numpy as np

BOHR = 1.8897261258369282

B, N, NK = 2, 4096, 4096
NCORES = 8
KSH = NK // 4          # k-vectors per core
NCH = N // 128         # 32 n-chunks
CW = 2 * KSH           # cs chunk width: [sin | cos]
NSL = CW // 128        # 16 k-slices per chunk
MAGIC = 12582912.0     # 1.5 * 2**23: fp32 round-to-nearest-integer
NEG2PI = -6.28318452835083  # two fp32 ulps below 2*pi

_PROG = {}


def _build_program(reps: int = 1, stage: str = 'full', n_assist: int = 14,
                   mm_bufs: int = 2, rn_bufs: int = 2, cs_bufs: int = 3,
                   extr_act: int = 2, p2_act: int = 1):
    import concourse.bass as bass
    import concourse.bacc as bacc
    import concourse.tile as tile
    import concourse.mybir as mybir

    F32 = mybir.dt.float32
    F32R = mybir.dt.float32r
    F16 = mybir.dt.float16
    U16 = mybir.dt.uint16
    ADD = mybir.AluOpType.add
    SUB = mybir.AluOpType.subtract
    MULT = mybir.AluOpType.mult
    AND = mybir.AluOpType.bitwise_and
    SIN = mybir.ActivationFunctionType.Sin
    COPY = mybir.ActivationFunctionType.Copy

    nc = bacc.Bacc(trn_type="TRN2", target_bir_lowering=False, debug=False)

    coordsT_d = nc.dram_tensor("coordsT", [3, N], F32, kind="ExternalInput").ap()
    qT_d = nc.dram_tensor("qT", [128, NCH], F32, kind="ExternalInput").ap()
    kmodT_d = nc.dram_tensor("kmodT", [3, KSH], F32, kind="ExternalInput").ap()
    efT_d = nc.dram_tensor("efT", [128, NSL], F32, kind="ExternalInput").ap()
    recip_d = nc.dram_tensor("recip", [128, NCH], F32, kind="ExternalOutput").ap()

    # chunks whose rounding runs on the scalar engine (DVE/ACT balance)
    ASSIST = {int((i + 0.5) * NCH / n_assist) for i in range(n_assist)}

    with tile.TileContext(nc) as tc, ExitStack() as ctx:
        const = ctx.enter_context(tc.tile_pool(name="const", bufs=1))
        pu = ctx.enter_context(tc.tile_pool(name="pu", bufs=3, space="PSUM"))
        pacc = ctx.enter_context(tc.tile_pool(name="pacc", bufs=1, space="PSUM"))
        wk_rn = ctx.enter_context(tc.tile_pool(name="wk_rn", bufs=rn_bufs))
        wk_mm = ctx.enter_context(tc.tile_pool(name="wk_mm", bufs=mm_bufs))
        wk_cs = ctx.enter_context(tc.tile_pool(name="wk_cs", bufs=cs_bufs))
        wk_out = ctx.enter_context(tc.tile_pool(name="wk_out", bufs=1))

        # ---- load inputs (first chunk's dependencies in tiny pieces first) ----
        cts = const.tile([3, N], F32R)
        kmod = const.tile([3, KSH], F32R, name="kmod")
        nc.sync.dma_start(out=kmod[:, 0:512], in_=kmodT_d[:, 0:512].bitcast(F32R))
        nc.scalar.dma_start(out=cts[:, 0:384], in_=coordsT_d[:, 0:384].bitcast(F32R))
        nc.sync.dma_start(out=kmod[:, 512:1024],
                          in_=kmodT_d[:, 512:1024].bitcast(F32R))
        nc.sync.dma_start(out=cts[:, 384:1024],
                          in_=coordsT_d[:, 384:1024].bitcast(F32R))
        for hh in range(1024, N, 1024):
            nc.sync.dma_start(
                out=cts[:, hh:hh + 1024],
                in_=coordsT_d[:, hh:hh + 1024].bitcast(F32R),
            )
        qt = const.tile([128, NCH], F32)
        nc.sync.dma_start(out=qt[:, :], in_=qT_d)
        efT = const.tile([128, NSL], F32)
        nc.sync.dma_start(out=efT[:, :], in_=efT_d)
        qt16 = const.tile([128, NCH], F16)
        one1 = const.tile([1, 1], F32)

        # persistent: the transposed trig store and the S quadrant rows
        csT = const.tile([128, NSL, N], F16)   # [k-in-slice][slice j][n]
        sgrid = pacc.tile([128, 512], F32, tag="sg")

        def emit_u(t):
            u_ps = pu.tile([128, KSH], F32, tag="u", name=f"u{t}")
            for hh in range(0, KSH, 512):
                nc.tensor.matmul(
                    u_ps[:, hh:hh + 512],
                    lhsT=cts[:, 128 * t:128 * (t + 1)],
                    rhs=kmod[:, hh:hh + 512],
                    start=True, stop=True,
                )
            return u_ps

        us = {t: emit_u(t) for t in range(3)}
        for s in range(NCH // 2):
            mmp = wk_mm.tile([128, 2 * CW], F16, tag="mm", name=f"mm{s}")
            for c in (0, 1):
                t = 2 * s + c
                u_ps = us.pop(t)
                fsl = mmp[:, CW * c:CW * c + KSH]            # f (sin half)
                hsl = mmp[:, CW * c + KSH:CW * (c + 1)]      # h (cos half)
                rn = wk_rn.tile([128, KSH], F32, tag="rn", name=f"rn{t}")
                if t in ASSIST and stage != 'rr0':
                    # ACT: v = u + M (rounds); DVE stt: (v - M) - u
                    nc.scalar.activation(rn[:, :], u_ps[:, :], COPY,
                                         bias=MAGIC, scale=1.0)
                    s0 = MAGIC
                else:
                    nc.vector.tensor_scalar(
                        out=rn[:, :], in0=u_ps[:, :],
                        scalar1=MAGIC, scalar2=MAGIC, op0=ADD, op1=SUB,
                    )
                    s0 = 0.0
                nc.vector.scalar_tensor_tensor(
                    out=fsl, in0=rn[:, :], scalar=s0, in1=u_ps[:, :],
                    op0=ADD if s0 == 0.0 else SUB, op1=SUB,
                )
                # g = |f| via uint16 AND; h = g - 1/4 on the pool engine
                nc.vector.tensor_scalar(
                    out=hsl.bitcast(U16), in0=fsl.bitcast(U16),
                    scalar1=0x7FFF, scalar2=None, op0=AND,
                )
                nc.gpsimd.tensor_scalar(
                    out=hsl, in0=hsl, scalar1=0.25, scalar2=None, op0=SUB,
                )
                tn = t + 3
                if tn < NCH:
                    us[tn] = emit_u(tn)
            if s == 0:
                nc.vector.tensor_copy(qt16[:, :], qt[:, :])
                nc.vector.memset(one1[:, :], 1.0)
            if stage == 'rr':
                continue
            # sin(theta) = Sin(-2pi f); cos(theta) = Sin(-2pi h)
            csp = wk_cs.tile([128, 2 * CW], F16, tag="cs", name=f"cs{s}")
            if s == NCH // 2 - 1:
                nc.scalar.activation(csp[:, 0:CW], mmp[:, 0:CW], SIN,
                                     bias=0.0, scale=NEG2PI)
            else:
                nc.scalar.activation(csp[:, :], mmp[:, :], SIN,
                                     bias=0.0, scale=NEG2PI)
            for c in (0, 1):
                t = 2 * s + c
                if s == NCH // 2 - 1 and c == 1:
                    nc.scalar.activation(csp[:, CW:], mmp[:, CW:], SIN,
                                         bias=0.0, scale=NEG2PI)
                for qd in range(4):
                    nc.tensor.matmul(
                        sgrid[32 * qd:32 * qd + 1, 0:512],
                        lhsT=qt16[:, t:t + 1],
                        rhs=csp[:, CW * c + 512 * qd:CW * c + 512 * (qd + 1)],
                        start=(t == 0), stop=(t == NCH - 1),
                        tile_position=(0, 32 * qd),
                    )
                if stage != 'act':
                    # csT[p, j, 128t + n] = csp[n, CW*c + 128j + p]
                    nc.sync.dma_start_transpose(
                        out=csT[:, :, 128 * t:128 * (t + 1)],
                        in_=csp[:, CW * c:CW * (c + 1)],
                    )

        if stage != 'full':
            zz = wk_out.tile([128, NCH], F32, name="zz")
            nc.vector.memset(zz[:, :], 0.0)
            nc.sync.dma_start(out=recip_d, in_=zz[:, :])
        else:
            # ---- S extraction: 4 quadrant rows -> SBUF (DVE + ACT) ----
            sgq0 = const.tile([1, 512], F32, name="sgq0")
            sgq1 = const.tile([1, 512], F32, name="sgq1")
            sgq2 = const.tile([1, 512], F32, name="sgq2")
            sgq3 = const.tile([1, 512], F32, name="sgq3")
            sgq = [sgq0, sgq1, sgq2, sgq3]
            wtp = pacc.tile([128, 512], F32, tag="wtp")
            for qd in range(4):
                if qd >= extr_act:
                    nc.vector.tensor_copy(sgq[qd][:, :],
                                          sgrid[32 * qd:32 * qd + 1, 0:512])
                else:
                    nc.scalar.copy(sgq[qd][:, :],
                                   sgrid[32 * qd:32 * qd + 1, 0:512])
                for aa in range(4):
                    j = 4 * qd + aa
                    nc.tensor.matmul(
                        wtp[:, j:j + 1],
                        lhsT=sgq[qd][0:1, 128 * aa:128 * (aa + 1)],
                        rhs=one1[:, :],
                        is_transpose=True, start=(j == 0), stop=(j == NSL - 1),
                    )
            wcolT = const.tile([128, NSL], F16, tag="wcolT")
            nc.vector.tensor_tensor(
                out=wcolT[:, :], in0=wtp[:, 0:NSL], in1=efT[:, :], op=MULT,
            )

            # ---- pass 2: recip[n] = sum_j csT[:, j, n]^T wcolT[:, j] ----
            rr = wk_out.tile([128, NCH], F32, name="rr")
            for m in range(NCH // 2):
                rp = pu.tile([128, KSH], F32, tag="u", name=f"rp{m}")
                for c in (0, 1):
                    t = 2 * m + c
                    for j in range(NSL):
                        nc.tensor.matmul(
                            rp[:, 512 * c:512 * c + 1],
                            lhsT=csT[:, j, 128 * t:128 * (t + 1)],
                            rhs=wcolT[:, j:j + 1],
                            start=(j == 0), stop=(j == NSL - 1),
                        )
                if m % 4 < p2_act:
                    nc.scalar.copy(rr[:, 2 * m:2 * m + 2], rp[:, 0:KSH:512])
                else:
                    nc.vector.tensor_copy(rr[:, 2 * m:2 * m + 2],
                                          rp[:, 0:KSH:512])
            nc.sync.dma_start(out=recip_d, in_=rr[:, :])

    nc.compile()
    return nc


def _get_prog(reps: int = 1, stage: str = "full", **kw):
    key = (reps, stage, tuple(sorted(kw.items())))
    if key not in _PROG:
        _PROG[key] = _build_program(reps, stage, **kw)
    return _PROG[key]


def _make_in_maps(coords, q, cell_inv, kvec, expfac):
    in_maps = []
    for c in range(NCORES):
        b, ks = divmod(c, NCORES // B)
        sl = slice(KSH * ks, KSH * (ks + 1))
        # efT[p, j] = expfac_shard[(128j + p) mod 1024]
        ef = np.asarray(expfac[sl], dtype=np.float32).reshape(8, 128).T  # [128, 8]
        efT = np.concatenate([ef, ef], axis=1)                           # [128, 16]
        in_maps.append({
            "coordsT": np.ascontiguousarray(coords[b].T, dtype=np.float32),
            "qT": np.ascontiguousarray(q[b].reshape(NCH, 128).T, dtype=np.float32),
            "kmodT": np.ascontiguousarray(
                cell_inv.astype(np.float32).T @ kvec[sl].T.astype(np.float32)),
            "efT": np.ascontiguousarray(efT),
        })
    return in_maps


def _finalize(results, q, volume, bewald):
    recip = np.zeros((B, N), np.float32)
    for c in range(NCORES):
        b = c // (NCORES // B)
        recip[b] += results[c]["recip"].T.reshape(-1)
    scale1 = np.float32(BOHR / (math.pi * float(volume[0])))
    scale2 = np.float32(2.0 * float(bewald[0]) * BOHR / math.sqrt(math.pi))
    phi = (recip * scale1 - q.astype(np.float32) * scale2).astype(np.float32)
    e = (np.float32(0.5) * q.astype(np.float32) * phi).astype(np.float32)
    return e, phi


def kernel(coords, q, cell_inv, kvec, expfac, volume, bewald):
    from concourse.bass_utils import run_bass_kernel_spmd

    nc = _get_prog()
    in_maps = _make_in_maps(coords, q, cell_inv, kvec, expfac)
    res = run_bass_kernel_spmd(nc, in_maps, list(range(NCORES))).results
    return _finalize(res, q, volume, bewald)


# revision 4
# speedup vs baseline: 1.5242x; 1.0021x over previous
"""Ewald reciprocal-space kernel for Trainium2 (8 NeuronCores, SPMD) — v3.

Math (per batch b):
    s        = cell_inv @ x          (fractional coords)
    theta    = 2*pi * (kvec . s)     (B, N, NK) phases
    S_re/S_im= sum_n q_n {cos,sin}(theta)          (structure factor)
    recip_n  = sum_k expfac_k (S_re cos + S_im sin)
    phi      = recip * BOHR/(pi*V) - q * 2*bewald*BOHR/sqrt(pi)
    returns (0.5*q*phi, phi)

Sharding: 8 cores = 2 batches x 4 k-shards (1024 k-vectors each). Each core
computes its full-N, shard-K contribution to recip with no collectives; host
sums the 4 shard partials per batch and applies the final affine.

Device pipeline per core (N=4096 as 32 chunks of 128 partitions):
  u  = x . kmod (kmod = Cinv^T k)       fp32r matmul into PSUM
  rn = (u + M) - M                      magic round; DVE or ACT-assisted
  f  = rn - u  in [-1/2, 1/2]           DVE scalar_tensor_tensor, fp16 out
  g  = |f|     (uint16 AND 0x7FFF)      DVE 4x-mode fp16
  h  = g - 1/4                          GPSIMD (pool), fp16
  sin(theta) = Sin(-2pi f), cos(theta) = Sin(-2pi h)   one ACT Sin per 2 chunks
  S rows: 4 PE matmuls/chunk, out [1, 512] parked at quadrant partition 32q
     (4 concurrent PSUM groups share one 2KB region on disjoint partitions)
  cs chunks DMA-transposed (xbar) into csT[k-slice partitions, n free]
  S extraction: 4 single-partition row copies (DVE+ACT), then 16 tiny PE
     transposes [1,128]->[128,1] into one PSUM group; w = efT * S^T (DVE)
  recip: free-size-1 weight-stationary PE matmuls (lhsT = csT block,
     rhs = w column), accumulated over 16 k-slices; PSUM ring reuse with
     per-pair copy-out, recip emitted as [128, 32]
"""

import math
from contextlib import ExitStack

import numpy as np

BOHR = 1.8897261258369282

B, N, NK = 2, 4096, 4096
NCORES = 8
KSH = NK // 4          # k-vectors per core
NCH = N // 128         # 32 n-chunks
CW = 2 * KSH           # cs chunk width: [sin | cos]
NSL = CW // 128        # 16 k-slices per chunk
MAGIC = 12582912.0     # 1.5 * 2**23: fp32 round-to-nearest-integer
NEG2PI = -6.28318452835083  # two fp32 ulps below 2*pi

_PROG = {}


def _build_program(reps: int = 1, stage: str = 'full', n_assist: int = 14,
                   mm_bufs: int = 2, rn_bufs: int = 2, cs_bufs: int = 3,
                   extr_act: int = 4, p2_act: int = 1):
    import concourse.bass as bass
    import concourse.bacc as bacc
    import concourse.tile as tile
    import concourse.mybir as mybir

    F32 = mybir.dt.float32
    F32R = mybir.dt.float32r
    F16 = mybir.dt.float16
    U16 = mybir.dt.uint16
    ADD = mybir.AluOpType.add
    SUB = mybir.AluOpType.subtract
    MULT = mybir.AluOpType.mult
    AND = mybir.AluOpType.bitwise_and
    SIN = mybir.ActivationFunctionType.Sin
    COPY = mybir.ActivationFunctionType.Copy

    nc = bacc.Bacc(trn_type="TRN2", target_bir_lowering=False, debug=False)

    coordsT_d = nc.dram_tensor("coordsT", [3, N], F32, kind="ExternalInput").ap()
    qT_d = nc.dram_tensor("qT", [128, NCH], F32, kind="ExternalInput").ap()
    kmodT_d = nc.dram_tensor("kmodT", [3, KSH], F32, kind="ExternalInput").ap()
    efT_d = nc.dram_tensor("efT", [128, NSL], F32, kind="ExternalInput").ap()
    recip_d = nc.dram_tensor("recip", [128, NCH], F32, kind="ExternalOutput").ap()

    # chunks whose rounding runs on the scalar engine (DVE/ACT balance)
    ASSIST = {int((i + 0.5) * NCH / n_assist) for i in range(n_assist)}

    with tile.TileContext(nc) as tc, ExitStack() as ctx:
        const = ctx.enter_context(tc.tile_pool(name="const", bufs=1))
        pu = ctx.enter_context(tc.tile_pool(name="pu", bufs=3, space="PSUM"))
        pacc = ctx.enter_context(tc.tile_pool(name="pacc", bufs=1, space="PSUM"))
        wk_rn = ctx.enter_context(tc.tile_pool(name="wk_rn", bufs=rn_bufs))
        wk_mm = ctx.enter_context(tc.tile_pool(name="wk_mm", bufs=mm_bufs))
        wk_cs = ctx.enter_context(tc.tile_pool(name="wk_cs", bufs=cs_bufs))
        wk_out = ctx.enter_context(tc.tile_pool(name="wk_out", bufs=1))

        # ---- load inputs (first chunk's dependencies in tiny pieces first) ----
        cts = const.tile([3, N], F32R)
        kmod = const.tile([3, KSH], F32R, name="kmod")
        nc.sync.dma_start(out=kmod[:, 0:512], in_=kmodT_d[:, 0:512].bitcast(F32R))
        nc.scalar.dma_start(out=cts[:, 0:384], in_=coordsT_d[:, 0:384].bitcast(F32R))
        nc.sync.dma_start(out=kmod[:, 512:1024],
                          in_=kmodT_d[:, 512:1024].bitcast(F32R))
        nc.sync.dma_start(out=cts[:, 384:1024],
                          in_=coordsT_d[:, 384:1024].bitcast(F32R))
        for hh in range(1024, N, 1024):
            nc.sync.dma_start(
                out=cts[:, hh:hh + 1024],
                in_=coordsT_d[:, hh:hh + 1024].bitcast(F32R),
            )
        qt = const.tile([128, NCH], F32)
        nc.sync.dma_start(out=qt[:, :], in_=qT_d)
        efT = const.tile([128, NSL], F32)
        nc.sync.dma_start(out=efT[:, :], in_=efT_d)
        qt16 = const.tile([128, NCH], F16)
        one1 = const.tile([1, 1], F32)

        # persistent: the transposed trig store and the S quadrant rows
        csT = const.tile([128, NSL, N], F16)   # [k-in-slice][slice j][n]
        sgrid = pacc.tile([128, 512], F32, tag="sg")

        def emit_u(t):
            u_ps = pu.tile([128, KSH], F32, tag="u", name=f"u{t}")
            for hh in range(0, KSH, 512):
                nc.tensor.matmul(
                    u_ps[:, hh:hh + 512],
                    lhsT=cts[:, 128 * t:128 * (t + 1)],
                    rhs=kmod[:, hh:hh + 512],
                    start=True, stop=True,
                )
            return u_ps

        us = {t: emit_u(t) for t in range(3)}
        for s in range(NCH // 2):
            mmp = wk_mm.tile([128, 2 * CW], F16, tag="mm", name=f"mm{s}")
            for c in (0, 1):
                t = 2 * s + c
                u_ps = us.pop(t)
                fsl = mmp[:, CW * c:CW * c + KSH]            # f (sin half)
                hsl = mmp[:, CW * c + KSH:CW * (c + 1)]      # h (cos half)
                rn = wk_rn.tile([128, KSH], F32, tag="rn", name=f"rn{t}")
                if t in ASSIST and stage != 'rr0':
                    # ACT: v = u + M (rounds); DVE stt: (v - M) - u
                    nc.scalar.activation(rn[:, :], u_ps[:, :], COPY,
                                         bias=MAGIC, scale=1.0)
                    s0 = MAGIC
                else:
                    nc.vector.tensor_scalar(
                        out=rn[:, :], in0=u_ps[:, :],
                        scalar1=MAGIC, scalar2=MAGIC, op0=ADD, op1=SUB,
                    )
                    s0 = 0.0
                nc.vector.scalar_tensor_tensor(
                    out=fsl, in0=rn[:, :], scalar=s0, in1=u_ps[:, :],
                    op0=ADD if s0 == 0.0 else SUB, op1=SUB,
                )
                # g = |f| via uint16 AND; h = g - 1/4 on the pool engine
                nc.vector.tensor_scalar(
                    out=hsl.bitcast(U16), in0=fsl.bitcast(U16),
                    scalar1=0x7FFF, scalar2=None, op0=AND,
                )
                nc.gpsimd.tensor_scalar(
                    out=hsl, in0=hsl, scalar1=0.25, scalar2=None, op0=SUB,
                )
                tn = t + 3
                if tn < NCH:
                    us[tn] = emit_u(tn)
            if s == 0:
                nc.vector.tensor_copy(qt16[:, :], qt[:, :])
                nc.vector.memset(one1[:, :], 1.0)
            if stage == 'rr':
                continue
            # sin(theta) = Sin(-2pi f); cos(theta) = Sin(-2pi h)
            csp = wk_cs.tile([128, 2 * CW], F16, tag="cs", name=f"cs{s}")
            if s == NCH // 2 - 1:
                nc.scalar.activation(csp[:, 0:CW], mmp[:, 0:CW], SIN,
                                     bias=0.0, scale=NEG2PI)
            else:
                nc.scalar.activation(csp[:, :], mmp[:, :], SIN,
                                     bias=0.0, scale=NEG2PI)
            for c in (0, 1):
                t = 2 * s + c
                if s == NCH // 2 - 1 and c == 1:
                    nc.scalar.activation(csp[:, CW:], mmp[:, CW:], SIN,
                                         bias=0.0, scale=NEG2PI)
                for qd in range(4):
                    nc.tensor.matmul(
                        sgrid[32 * qd:32 * qd + 1, 0:512],
                        lhsT=qt16[:, t:t + 1],
                        rhs=csp[:, CW * c + 512 * qd:CW * c + 512 * (qd + 1)],
                        start=(t == 0), stop=(t == NCH - 1),
                        tile_position=(0, 32 * qd),
                    )
                if stage != 'act':
                    # csT[p, j, 128t + n] = csp[n, CW*c + 128j + p]
                    nc.sync.dma_start_transpose(
                        out=csT[:, :, 128 * t:128 * (t + 1)],
                        in_=csp[:, CW * c:CW * (c + 1)],
                    )

        if stage != 'full':
            zz = wk_out.tile([128, NCH], F32, name="zz")
            nc.vector.memset(zz[:, :], 0.0)
            nc.sync.dma_start(out=recip_d, in_=zz[:, :])
        else:
            # ---- S extraction: 4 quadrant rows -> SBUF (DVE + ACT) ----
            sgq0 = const.tile([1, 512], F32, name="sgq0")
            sgq1 = const.tile([1, 512], F32, name="sgq1")
            sgq2 = const.tile([1, 512], F32, name="sgq2")
            sgq3 = const.tile([1, 512], F32, name="sgq3")
            sgq = [sgq0, sgq1, sgq2, sgq3]
            wtp = pacc.tile([128, 512], F32, tag="wtp")
            for qd in range(4):
                if qd >= extr_act:
                    nc.vector.tensor_copy(sgq[qd][:, :],
                                          sgrid[32 * qd:32 * qd + 1, 0:512])
                else:
                    nc.scalar.copy(sgq[qd][:, :],
                                   sgrid[32 * qd:32 * qd + 1, 0:512])
                for aa in range(4):
                    j = 4 * qd + aa
                    nc.tensor.matmul(
                        wtp[:, j:j + 1],
                        lhsT=sgq[qd][0:1, 128 * aa:128 * (aa + 1)],
                        rhs=one1[:, :],
                        is_transpose=True, start=(j == 0), stop=(j == NSL - 1),
                    )
            wcolT = const.tile([128, NSL], F16, tag="wcolT")
            nc.vector.tensor_tensor(
                out=wcolT[:, :], in0=wtp[:, 0:NSL], in1=efT[:, :], op=MULT,
            )

            # ---- pass 2: recip[n] = sum_j csT[:, j, n]^T wcolT[:, j] ----
            rr = wk_out.tile([128, NCH], F32, name="rr")
            for m in range(NCH // 2):
                rp = pu.tile([128, KSH], F32, tag="u", name=f"rp{m}")
                for c in (0, 1):
                    t = 2 * m + c
                    for j in range(NSL):
                        nc.tensor.matmul(
                            rp[:, 512 * c:512 * c + 1],
                            lhsT=csT[:, j, 128 * t:128 * (t + 1)],
                            rhs=wcolT[:, j:j + 1],
                            start=(j == 0), stop=(j == NSL - 1),
                        )
                if m % 4 < p2_act:
                    nc.scalar.copy(rr[:, 2 * m:2 * m + 2], rp[:, 0:KSH:512])
                else:
                    nc.vector.tensor_copy(rr[:, 2 * m:2 * m + 2],
                                          rp[:, 0:KSH:512])
            nc.sync.dma_start(out=recip_d, in_=rr[:, :])

    nc.compile()
    return nc


def _get_prog(reps: int = 1, stage: str = "full", **kw):
    key = (reps, stage, tuple(sorted(kw.items())))
    if key not in _PROG:
        _PROG[key] = _build_program(reps, stage, **kw)
    return _PROG[key]


def _make_in_maps(coords, q, cell_inv, kvec, expfac):
    in_maps = []
    for c in range(NCORES):
        b, ks = divmod(c, NCORES // B)
        sl = slice(KSH * ks, KSH * (ks + 1))
        # efT[p, j] = expfac_shard[(128j + p) mod 1024]
        ef = np.asarray(expfac[sl], dtype=np.float32).reshape(8, 128).T  # [128, 8]
        efT = np.concatenate([ef, ef], axis=1)                           # [128, 16]
        in_maps.append({
            "coordsT": np.ascontiguousarray(coords[b].T, dtype=np.float32),
            "qT": np.ascontiguousarray(q[b].reshape(NCH, 128).T, dtype=np.float32),
            "kmodT": np.ascontiguousarray(
                cell_inv.astype(np.float32).T @ kvec[sl].T.astype(np.float32)),
            "efT": np.ascontiguousarray(efT),
        })
    return in_maps


def _finalize(results, q, volume, bewald):
    recip = np.zeros((B, N), np.float32)
    for c in range(NCORES):
        b = c // (NCORES // B)
        recip[b] += results[c]["recip"].T.reshape(-1)
    scale1 = np.float32(BOHR / (math.pi * float(volume[0])))
    scale2 = np.float32(2.0 * float(bewald[0]) * BOHR / math.sqrt(math.pi))
    phi = (recip * scale1 - q.astype(np.float32) * scale2).astype(np.float32)
    e = (np.float32(0.5) * q.astype(np.float32) * phi).astype(np.float32)
    return e, phi


def kernel(coords, q, cell_inv, kvec, expfac, volume, bewald):
    from concourse.bass_utils import run_bass_kernel_spmd

    nc = _get_prog()
    in_maps = _make_in_maps(coords, q, cell_inv, kvec, expfac)
    res = run_bass_kernel_spmd(nc, in_maps, list(range(NCORES))).results
    return _finalize(res, q, volume, bewald)


# revision 5
# speedup vs baseline: 1.5317x; 1.0049x over previous
"""Ewald reciprocal-space kernel for Trainium2 (8 NeuronCores, SPMD) — v3.

Math (per batch b):
    s        = cell_inv @ x          (fractional coords)
    theta    = 2*pi * (kvec . s)     (B, N, NK) phases
    S_re/S_im= sum_n q_n {cos,sin}(theta)          (structure factor)
    recip_n  = sum_k expfac_k (S_re cos + S_im sin)
    phi      = recip * BOHR/(pi*V) - q * 2*bewald*BOHR/sqrt(pi)
    returns (0.5*q*phi, phi)

Sharding: 8 cores = 2 batches x 4 k-shards (1024 k-vectors each). Each core
computes its full-N, shard-K contribution to recip with no collectives; host
sums the 4 shard partials per batch and applies the final affine.

Device pipeline per core (N=4096 as 32 chunks of 128 partitions):
  u  = x . kmod (kmod = Cinv^T k)       fp32r matmul into PSUM
  rn = (u + M) - M                      magic round; DVE or ACT-assisted
  f  = rn - u  in [-1/2, 1/2]           DVE scalar_tensor_tensor, fp16 out
  g  = |f|     (uint16 AND 0x7FFF)      DVE 4x-mode fp16
  h  = g - 1/4                          GPSIMD (pool), fp16
  sin(theta) = Sin(-2pi f), cos(theta) = Sin(-2pi h)   one ACT Sin per 2 chunks
  S rows: 4 PE matmuls/chunk, out [1, 512] parked at quadrant partition 32q
     (4 concurrent PSUM groups share one 2KB region on disjoint partitions)
  cs chunks DMA-transposed (xbar) into csT[k-slice partitions, n free]
  S extraction: 4 single-partition row copies (DVE+ACT), then 16 tiny PE
     transposes [1,128]->[128,1] into one PSUM group; w = efT * S^T (DVE)
  recip: free-size-1 weight-stationary PE matmuls (lhsT = csT block,
     rhs = w column), accumulated over 16 k-slices; PSUM ring reuse with
     per-pair copy-out, recip emitted as [128, 32]
"""

import math
from contextlib import ExitStack

import numpy as np

BOHR = 1.8897261258369282

B, N, NK = 2, 4096, 4096
NCORES = 8
KSH = NK // 4          # k-vectors per core
NCH = N // 128         # 32 n-chunks
CW = 2 * KSH           # cs chunk width: [sin | cos]
NSL = CW // 128        # 16 k-slices per chunk
MAGIC = 12582912.0     # 1.5 * 2**23: fp32 round-to-nearest-integer
NEG2PI = -6.28318452835083  # two fp32 ulps below 2*pi

_PROG = {}


def _build_program(reps: int = 1, stage: str = 'full', n_assist: int = 14,
                   mm_bufs: int = 2, rn_bufs: int = 2, cs_bufs: int = 3,
                   extr_act: int = 4, p2_act: int = 1, assist_last: int = NCH):
    import concourse.bass as bass
    import concourse.bacc as bacc
    import concourse.tile as tile
    import concourse.mybir as mybir

    F32 = mybir.dt.float32
    F32R = mybir.dt.float32r
    F16 = mybir.dt.float16
    U16 = mybir.dt.uint16
    ADD = mybir.AluOpType.add
    SUB = mybir.AluOpType.subtract
    MULT = mybir.AluOpType.mult
    AND = mybir.AluOpType.bitwise_and
    SIN = mybir.ActivationFunctionType.Sin
    COPY = mybir.ActivationFunctionType.Copy

    nc = bacc.Bacc(trn_type="TRN2", target_bir_lowering=False, debug=False)

    coordsT_d = nc.dram_tensor("coordsT", [3, N], F32, kind="ExternalInput").ap()
    qT_d = nc.dram_tensor("qT", [128, NCH], F32, kind="ExternalInput").ap()
    kmodT_d = nc.dram_tensor("kmodT", [3, KSH], F32, kind="ExternalInput").ap()
    efT_d = nc.dram_tensor("efT", [128, NSL], F32, kind="ExternalInput").ap()
    recip_d = nc.dram_tensor("recip", [128, NCH], F32, kind="ExternalOutput").ap()

    # chunks whose rounding runs on the scalar engine (DVE/ACT balance);
    # assist_last bounds the last assisted chunk so the ACT tail is Sin-only
    ASSIST = {int((i + 0.5) * assist_last / n_assist) for i in range(n_assist)}

    with tile.TileContext(nc) as tc, ExitStack() as ctx:
        const = ctx.enter_context(tc.tile_pool(name="const", bufs=1))
        pu = ctx.enter_context(tc.tile_pool(name="pu", bufs=3, space="PSUM"))
        pacc = ctx.enter_context(tc.tile_pool(name="pacc", bufs=1, space="PSUM"))
        wk_rn = ctx.enter_context(tc.tile_pool(name="wk_rn", bufs=rn_bufs))
        wk_mm = ctx.enter_context(tc.tile_pool(name="wk_mm", bufs=mm_bufs))
        wk_cs = ctx.enter_context(tc.tile_pool(name="wk_cs", bufs=cs_bufs))
        wk_out = ctx.enter_context(tc.tile_pool(name="wk_out", bufs=1))

        # ---- load inputs (first chunk's dependencies in tiny pieces first) ----
        cts = const.tile([3, N], F32R)
        kmod = const.tile([3, KSH], F32R, name="kmod")
        nc.sync.dma_start(out=kmod[:, 0:512], in_=kmodT_d[:, 0:512].bitcast(F32R))
        nc.scalar.dma_start(out=cts[:, 0:384], in_=coordsT_d[:, 0:384].bitcast(F32R))
        nc.sync.dma_start(out=kmod[:, 512:1024],
                          in_=kmodT_d[:, 512:1024].bitcast(F32R))
        nc.sync.dma_start(out=cts[:, 384:1024],
                          in_=coordsT_d[:, 384:1024].bitcast(F32R))
        for hh in range(1024, N, 1024):
            nc.sync.dma_start(
                out=cts[:, hh:hh + 1024],
                in_=coordsT_d[:, hh:hh + 1024].bitcast(F32R),
            )
        qt = const.tile([128, NCH], F32)
        nc.sync.dma_start(out=qt[:, :], in_=qT_d)
        efT = const.tile([128, NSL], F32)
        nc.sync.dma_start(out=efT[:, :], in_=efT_d)
        qt16 = const.tile([128, NCH], F16)
        one1 = const.tile([1, 1], F32)

        # persistent: the transposed trig store and the S quadrant rows
        csT = const.tile([128, NSL, N], F16)   # [k-in-slice][slice j][n]
        sgrid = pacc.tile([128, 512], F32, tag="sg")

        def emit_u(t):
            u_ps = pu.tile([128, KSH], F32, tag="u", name=f"u{t}")
            for hh in range(0, KSH, 512):
                nc.tensor.matmul(
                    u_ps[:, hh:hh + 512],
                    lhsT=cts[:, 128 * t:128 * (t + 1)],
                    rhs=kmod[:, hh:hh + 512],
                    start=True, stop=True,
                )
            return u_ps

        us = {t: emit_u(t) for t in range(3)}
        for s in range(NCH // 2):
            mmp = wk_mm.tile([128, 2 * CW], F16, tag="mm", name=f"mm{s}")
            for c in (0, 1):
                t = 2 * s + c
                u_ps = us.pop(t)
                fsl = mmp[:, CW * c:CW * c + KSH]            # f (sin half)
                hsl = mmp[:, CW * c + KSH:CW * (c + 1)]      # h (cos half)
                rn = wk_rn.tile([128, KSH], F32, tag="rn", name=f"rn{t}")
                if t in ASSIST and stage != 'rr0':
                    # ACT: v = u + M (rounds); DVE stt: (v - M) - u
                    nc.scalar.activation(rn[:, :], u_ps[:, :], COPY,
                                         bias=MAGIC, scale=1.0)
                    s0 = MAGIC
                else:
                    nc.vector.tensor_scalar(
                        out=rn[:, :], in0=u_ps[:, :],
                        scalar1=MAGIC, scalar2=MAGIC, op0=ADD, op1=SUB,
                    )
                    s0 = 0.0
                nc.vector.scalar_tensor_tensor(
                    out=fsl, in0=rn[:, :], scalar=s0, in1=u_ps[:, :],
                    op0=ADD if s0 == 0.0 else SUB, op1=SUB,
                )
                # g = |f| via uint16 AND; h = g - 1/4 on the pool engine
                nc.vector.tensor_scalar(
                    out=hsl.bitcast(U16), in0=fsl.bitcast(U16),
                    scalar1=0x7FFF, scalar2=None, op0=AND,
                )
                nc.gpsimd.tensor_scalar(
                    out=hsl, in0=hsl, scalar1=0.25, scalar2=None, op0=SUB,
                )
                tn = t + 3
                if tn < NCH:
                    us[tn] = emit_u(tn)
            if s == 0:
                nc.vector.tensor_copy(qt16[:, :], qt[:, :])
                nc.vector.memset(one1[:, :], 1.0)
            if stage == 'rr':
                continue
            # sin(theta) = Sin(-2pi f); cos(theta) = Sin(-2pi h)
            csp = wk_cs.tile([128, 2 * CW], F16, tag="cs", name=f"cs{s}")
            if s in (0, NCH // 2 - 1):
                nc.scalar.activation(csp[:, 0:CW], mmp[:, 0:CW], SIN,
                                     bias=0.0, scale=NEG2PI)
            else:
                nc.scalar.activation(csp[:, :], mmp[:, :], SIN,
                                     bias=0.0, scale=NEG2PI)
            for c in (0, 1):
                t = 2 * s + c
                if s in (0, NCH // 2 - 1) and c == 1:
                    nc.scalar.activation(csp[:, CW:], mmp[:, CW:], SIN,
                                         bias=0.0, scale=NEG2PI)
                for qd in range(4):
                    nc.tensor.matmul(
                        sgrid[32 * qd:32 * qd + 1, 0:512],
                        lhsT=qt16[:, t:t + 1],
                        rhs=csp[:, CW * c + 512 * qd:CW * c + 512 * (qd + 1)],
                        start=(t == 0), stop=(t == NCH - 1),
                        tile_position=(0, 32 * qd),
                    )
                if stage != 'act':
                    # csT[p, j, 128t + n] = csp[n, CW*c + 128j + p]
                    nc.sync.dma_start_transpose(
                        out=csT[:, :, 128 * t:128 * (t + 1)],
                        in_=csp[:, CW * c:CW * (c + 1)],
                    )

        if stage != 'full':
            zz = wk_out.tile([128, NCH], F32, name="zz")
            nc.vector.memset(zz[:, :], 0.0)
            nc.sync.dma_start(out=recip_d, in_=zz[:, :])
        else:
            # ---- S extraction: 4 quadrant rows -> SBUF (DVE + ACT) ----
            sgq0 = const.tile([1, 512], F32, name="sgq0")
            sgq1 = const.tile([1, 512], F32, name="sgq1")
            sgq2 = const.tile([1, 512], F32, name="sgq2")
            sgq3 = const.tile([1, 512], F32, name="sgq3")
            sgq = [sgq0, sgq1, sgq2, sgq3]
            wtp = pacc.tile([128, 512], F32, tag="wtp")
            for qd in range(4):
                if qd >= extr_act:
                    nc.vector.tensor_copy(sgq[qd][:, :],
                                          sgrid[32 * qd:32 * qd + 1, 0:512])
                else:
                    nc.scalar.copy(sgq[qd][:, :],
                                   sgrid[32 * qd:32 * qd + 1, 0:512])
                for aa in range(4):
                    j = 4 * qd + aa
                    nc.tensor.matmul(
                        wtp[:, j:j + 1],
                        lhsT=sgq[qd][0:1, 128 * aa:128 * (aa + 1)],
                        rhs=one1[:, :],
                        is_transpose=True, start=(j == 0), stop=(j == NSL - 1),
                    )
            wcolT = const.tile([128, NSL], F16, tag="wcolT")
            nc.vector.tensor_tensor(
                out=wcolT[:, :], in0=wtp[:, 0:NSL], in1=efT[:, :], op=MULT,
            )

            # ---- pass 2: recip[n] = sum_j csT[:, j, n]^T wcolT[:, j] ----
            rr = wk_out.tile([128, NCH], F32, name="rr")
            for m in range(NCH // 2):
                rp = pu.tile([128, KSH], F32, tag="u", name=f"rp{m}")
                for c in (0, 1):
                    t = 2 * m + c
                    for j in range(NSL):
                        nc.tensor.matmul(
                            rp[:, 512 * c:512 * c + 1],
                            lhsT=csT[:, j, 128 * t:128 * (t + 1)],
                            rhs=wcolT[:, j:j + 1],
                            start=(j == 0), stop=(j == NSL - 1),
                        )
                if m % 4 < p2_act:
                    nc.scalar.copy(rr[:, 2 * m:2 * m + 2], rp[:, 0:KSH:512])
                else:
                    nc.vector.tensor_copy(rr[:, 2 * m:2 * m + 2],
                                          rp[:, 0:KSH:512])
            nc.sync.dma_start(out=recip_d, in_=rr[:, :])

    nc.compile()
    return nc


def _get_prog(reps: int = 1, stage: str = "full", **kw):
    key = (reps, stage, tuple(sorted(kw.items())))
    if key not in _PROG:
        _PROG[key] = _build_program(reps, stage, **kw)
    return _PROG[key]


def _make_in_maps(coords, q, cell_inv, kvec, expfac):
    in_maps = []
    for c in range(NCORES):
        b, ks = divmod(c, NCORES // B)
        sl = slice(KSH * ks, KSH * (ks + 1))
        # efT[p, j] = expfac_shard[(128j + p) mod 1024]
        ef = np.asarray(expfac[sl], dtype=np.float32).reshape(8, 128).T  # [128, 8]
        efT = np.concatenate([ef, ef], axis=1)                           # [128, 16]
        in_maps.append({
            "coordsT": np.ascontiguousarray(coords[b].T, dtype=np.float32),
            "qT": np.ascontiguousarray(q[b].reshape(NCH, 128).T, dtype=np.float32),
            "kmodT": np.ascontiguousarray(
                cell_inv.astype(np.float32).T @ kvec[sl].T.astype(np.float32)),
            "efT": np.ascontiguousarray(efT),
        })
    return in_maps


def _finalize(results, q, volume, bewald):
    recip = np.zeros((B, N), np.float32)
    for c in range(NCORES):
        b = c // (NCORES // B)
        recip[b] += results[c]["recip"].T.reshape(-1)
    scale1 = np.float32(BOHR / (math.pi * float(volume[0])))
    scale2 = np.float32(2.0 * float(bewald[0]) * BOHR / math.sqrt(math.pi))
    phi = (recip * scale1 - q.astype(np.float32) * scale2).astype(np.float32)
    e = (np.float32(0.5) * q.astype(np.float32) * phi).astype(np.float32)
    return e, phi


def kernel(coords, q, cell_inv, kvec, expfac, volume, bewald):
    from concourse.bass_utils import run_bass_kernel_spmd

    nc = _get_prog()
    in_maps = _make_in_maps(coords, q, cell_inv, kvec, expfac)
    res = run_bass_kernel_spmd(nc, in_maps, list(range(NCORES))).results
    return _finalize(res, q, volume, bewald)


# revision 6
# speedup vs baseline: 1.5348x; 1.0020x over previous
"""Ewald reciprocal-space kernel for Trainium2 (8 NeuronCores, SPMD) — v3.

Math (per batch b):
    s        = cell_inv @ x          (fractional coords)
    theta    = 2*pi * (kvec . s)     (B, N, NK) phases
    S_re/S_im= sum_n q_n {cos,sin}(theta)          (structure factor)
    recip_n  = sum_k expfac_k (S_re cos + S_im sin)
    phi      = recip * BOHR/(pi*V) - q * 2*bewald*BOHR/sqrt(pi)
    returns (0.5*q*phi, phi)

Sharding: 8 cores = 2 batches x 4 k-shards (1024 k-vectors each). Each core
computes its full-N, shard-K contribution to recip with no collectives; host
sums the 4 shard partials per batch and applies the final affine.

Device pipeline per core (N=4096 as 32 chunks of 128 partitions):
  u  = x . kmod (kmod = Cinv^T k)       fp32r matmul into PSUM
  rn = (u + M) - M                      magic round; DVE or ACT-assisted
  f  = rn - u  in [-1/2, 1/2]           DVE scalar_tensor_tensor, fp16 out
  g  = |f|     (uint16 AND 0x7FFF)      DVE 4x-mode fp16
  h  = g - 1/4                          GPSIMD (pool), fp16
  sin(theta) = Sin(-2pi f), cos(theta) = Sin(-2pi h)   one ACT Sin per 2 chunks
  S rows: 4 PE matmuls/chunk, out [1, 512] parked at quadrant partition 32q
     (4 concurrent PSUM groups share one 2KB region on disjoint partitions)
  cs chunks DMA-transposed (xbar) into csT[k-slice partitions, n free]
  S extraction: 4 single-partition row copies (DVE+ACT), then 16 tiny PE
     transposes [1,128]->[128,1] into one PSUM group; w = efT * S^T (DVE)
  recip: free-size-1 weight-stationary PE matmuls (lhsT = csT block,
     rhs = w column), accumulated over 16 k-slices; PSUM ring reuse with
     per-pair copy-out, recip emitted as [128, 32]
"""

import math
from contextlib import ExitStack

import numpy as np

BOHR = 1.8897261258369282

B, N, NK = 2, 4096, 4096
NCORES = 8
KSH = NK // 4          # k-vectors per core
NCH = N // 128         # 32 n-chunks
CW = 2 * KSH           # cs chunk width: [sin | cos]
NSL = CW // 128        # 16 k-slices per chunk
MAGIC = 12582912.0     # 1.5 * 2**23: fp32 round-to-nearest-integer
NEG2PI = -6.28318452835083  # two fp32 ulps below 2*pi

_PROG = {}


def _build_program(reps: int = 1, stage: str = 'full', n_assist: int = 14,
                   mm_bufs: int = 3, rn_bufs: int = 2, cs_bufs: int = 2,
                   extr_act: int = 4, p2_act: int = 1, assist_last: int = NCH):
    import concourse.bass as bass
    import concourse.bacc as bacc
    import concourse.tile as tile
    import concourse.mybir as mybir

    F32 = mybir.dt.float32
    F32R = mybir.dt.float32r
    F16 = mybir.dt.float16
    U16 = mybir.dt.uint16
    ADD = mybir.AluOpType.add
    SUB = mybir.AluOpType.subtract
    MULT = mybir.AluOpType.mult
    AND = mybir.AluOpType.bitwise_and
    SIN = mybir.ActivationFunctionType.Sin
    COPY = mybir.ActivationFunctionType.Copy

    nc = bacc.Bacc(trn_type="TRN2", target_bir_lowering=False, debug=False)

    coordsT_d = nc.dram_tensor("coordsT", [3, N], F32, kind="ExternalInput").ap()
    qT_d = nc.dram_tensor("qT", [128, NCH], F32, kind="ExternalInput").ap()
    kmodT_d = nc.dram_tensor("kmodT", [3, KSH], F32, kind="ExternalInput").ap()
    efT_d = nc.dram_tensor("efT", [128, NSL], F32, kind="ExternalInput").ap()
    recip_d = nc.dram_tensor("recip", [128, NCH], F32, kind="ExternalOutput").ap()

    # chunks whose rounding runs on the scalar engine (DVE/ACT balance);
    # assist_last bounds the last assisted chunk so the ACT tail is Sin-only
    ASSIST = {int((i + 0.5) * assist_last / n_assist) for i in range(n_assist)}

    with tile.TileContext(nc) as tc, ExitStack() as ctx:
        const = ctx.enter_context(tc.tile_pool(name="const", bufs=1))
        pu = ctx.enter_context(tc.tile_pool(name="pu", bufs=3, space="PSUM"))
        pacc = ctx.enter_context(tc.tile_pool(name="pacc", bufs=1, space="PSUM"))
        wk_rn = ctx.enter_context(tc.tile_pool(name="wk_rn", bufs=rn_bufs))
        wk_mm = ctx.enter_context(tc.tile_pool(name="wk_mm", bufs=mm_bufs))
        wk_cs = ctx.enter_context(tc.tile_pool(name="wk_cs", bufs=cs_bufs))
        wk_out = ctx.enter_context(tc.tile_pool(name="wk_out", bufs=1))

        # ---- load inputs (first chunk's dependencies in tiny pieces first) ----
        cts = const.tile([3, N], F32R)
        kmod = const.tile([3, KSH], F32R, name="kmod")
        nc.sync.dma_start(out=kmod[:, 0:512], in_=kmodT_d[:, 0:512].bitcast(F32R))
        nc.scalar.dma_start(out=cts[:, 0:384], in_=coordsT_d[:, 0:384].bitcast(F32R))
        nc.sync.dma_start(out=kmod[:, 512:1024],
                          in_=kmodT_d[:, 512:1024].bitcast(F32R))
        nc.sync.dma_start(out=cts[:, 384:1024],
                          in_=coordsT_d[:, 384:1024].bitcast(F32R))
        for hh in range(1024, N, 1024):
            nc.sync.dma_start(
                out=cts[:, hh:hh + 1024],
                in_=coordsT_d[:, hh:hh + 1024].bitcast(F32R),
            )
        qt = const.tile([128, NCH], F32)
        nc.sync.dma_start(out=qt[:, :], in_=qT_d)
        efT = const.tile([128, NSL], F32)
        nc.sync.dma_start(out=efT[:, :], in_=efT_d)
        qt16 = const.tile([128, NCH], F16)
        one1 = const.tile([1, 1], F32)

        # persistent: the transposed trig store and the S quadrant rows
        csT = const.tile([128, NSL, N], F16)   # [k-in-slice][slice j][n]
        sgrid = pacc.tile([128, 512], F32, tag="sg")

        def emit_u(t):
            u_ps = pu.tile([128, KSH], F32, tag="u", name=f"u{t}")
            for hh in range(0, KSH, 512):
                nc.tensor.matmul(
                    u_ps[:, hh:hh + 512],
                    lhsT=cts[:, 128 * t:128 * (t + 1)],
                    rhs=kmod[:, hh:hh + 512],
                    start=True, stop=True,
                )
            return u_ps

        us = {t: emit_u(t) for t in range(3)}
        for s in range(NCH // 2):
            mmp = wk_mm.tile([128, 2 * CW], F16, tag="mm", name=f"mm{s}")
            for c in (0, 1):
                t = 2 * s + c
                u_ps = us.pop(t)
                fsl = mmp[:, CW * c:CW * c + KSH]            # f (sin half)
                hsl = mmp[:, CW * c + KSH:CW * (c + 1)]      # h (cos half)
                rn = wk_rn.tile([128, KSH], F32, tag="rn", name=f"rn{t}")
                if t in ASSIST and stage != 'rr0':
                    # ACT: v = u + M (rounds); DVE stt: (v - M) - u
                    nc.scalar.activation(rn[:, :], u_ps[:, :], COPY,
                                         bias=MAGIC, scale=1.0)
                    s0 = MAGIC
                else:
                    nc.vector.tensor_scalar(
                        out=rn[:, :], in0=u_ps[:, :],
                        scalar1=MAGIC, scalar2=MAGIC, op0=ADD, op1=SUB,
                    )
                    s0 = 0.0
                nc.vector.scalar_tensor_tensor(
                    out=fsl, in0=rn[:, :], scalar=s0, in1=u_ps[:, :],
                    op0=ADD if s0 == 0.0 else SUB, op1=SUB,
                )
                # g = |f| via uint16 AND; h = g - 1/4 on the pool engine
                nc.vector.tensor_scalar(
                    out=hsl.bitcast(U16), in0=fsl.bitcast(U16),
                    scalar1=0x7FFF, scalar2=None, op0=AND,
                )
                nc.gpsimd.tensor_scalar(
                    out=hsl, in0=hsl, scalar1=0.25, scalar2=None, op0=SUB,
                )
                tn = t + 3
                if tn < NCH:
                    us[tn] = emit_u(tn)
            if s == 0:
                nc.vector.tensor_copy(qt16[:, :], qt[:, :])
                nc.vector.memset(one1[:, :], 1.0)
            if stage == 'rr':
                continue
            # sin(theta) = Sin(-2pi f); cos(theta) = Sin(-2pi h)
            csp = wk_cs.tile([128, 2 * CW], F16, tag="cs", name=f"cs{s}")
            if s in (0, NCH // 2 - 1):
                nc.scalar.activation(csp[:, 0:CW], mmp[:, 0:CW], SIN,
                                     bias=0.0, scale=NEG2PI)
            else:
                nc.scalar.activation(csp[:, :], mmp[:, :], SIN,
                                     bias=0.0, scale=NEG2PI)
            for c in (0, 1):
                t = 2 * s + c
                if s in (0, NCH // 2 - 1) and c == 1:
                    nc.scalar.activation(csp[:, CW:], mmp[:, CW:], SIN,
                                         bias=0.0, scale=NEG2PI)
                for qd in range(4):
                    nc.tensor.matmul(
                        sgrid[32 * qd:32 * qd + 1, 0:512],
                        lhsT=qt16[:, t:t + 1],
                        rhs=csp[:, CW * c + 512 * qd:CW * c + 512 * (qd + 1)],
                        start=(t == 0), stop=(t == NCH - 1),
                        tile_position=(0, 32 * qd),
                    )
                if stage != 'act':
                    # csT[p, j, 128t + n] = csp[n, CW*c + 128j + p]
                    nc.sync.dma_start_transpose(
                        out=csT[:, :, 128 * t:128 * (t + 1)],
                        in_=csp[:, CW * c:CW * (c + 1)],
                    )

        if stage != 'full':
            zz = wk_out.tile([128, NCH], F32, name="zz")
            nc.vector.memset(zz[:, :], 0.0)
            nc.sync.dma_start(out=recip_d, in_=zz[:, :])
        else:
            # ---- S extraction: 4 quadrant rows -> SBUF (DVE + ACT) ----
            sgq0 = const.tile([1, 512], F32, name="sgq0")
            sgq1 = const.tile([1, 512], F32, name="sgq1")
            sgq2 = const.tile([1, 512], F32, name="sgq2")
            sgq3 = const.tile([1, 512], F32, name="sgq3")
            sgq = [sgq0, sgq1, sgq2, sgq3]
            wtp = pacc.tile([128, 512], F32, tag="wtp")
            for qd in range(4):
                if qd >= extr_act:
                    nc.vector.tensor_copy(sgq[qd][:, :],
                                          sgrid[32 * qd:32 * qd + 1, 0:512])
                else:
                    nc.scalar.copy(sgq[qd][:, :],
                                   sgrid[32 * qd:32 * qd + 1, 0:512])
                for aa in range(4):
                    j = 4 * qd + aa
                    nc.tensor.matmul(
                        wtp[:, j:j + 1],
                        lhsT=sgq[qd][0:1, 128 * aa:128 * (aa + 1)],
                        rhs=one1[:, :],
                        is_transpose=True, start=(j == 0), stop=(j == NSL - 1),
                    )
            wcolT = const.tile([128, NSL], F16, tag="wcolT")
            nc.vector.tensor_tensor(
                out=wcolT[:, :], in0=wtp[:, 0:NSL], in1=efT[:, :], op=MULT,
            )

            # ---- pass 2: recip[n] = sum_j csT[:, j, n]^T wcolT[:, j] ----
            rr = wk_out.tile([128, NCH], F32, name="rr")
            for m in range(NCH // 2):
                rp = pu.tile([128, KSH], F32, tag="u", name=f"rp{m}")
                for c in (0, 1):
                    t = 2 * m + c
                    for j in range(NSL):
                        nc.tensor.matmul(
                            rp[:, 512 * c:512 * c + 1],
                            lhsT=csT[:, j, 128 * t:128 * (t + 1)],
                            rhs=wcolT[:, j:j + 1],
                            start=(j == 0), stop=(j == NSL - 1),
                        )
                if m % 4 < p2_act:
                    nc.scalar.copy(rr[:, 2 * m:2 * m + 2], rp[:, 0:KSH:512])
                else:
                    nc.vector.tensor_copy(rr[:, 2 * m:2 * m + 2],
                                          rp[:, 0:KSH:512])
            nc.sync.dma_start(out=recip_d, in_=rr[:, :])

    nc.compile()
    return nc


def _get_prog(reps: int = 1, stage: str = "full", **kw):
    key = (reps, stage, tuple(sorted(kw.items())))
    if key not in _PROG:
        _PROG[key] = _build_program(reps, stage, **kw)
    return _PROG[key]


def _make_in_maps(coords, q, cell_inv, kvec, expfac):
    in_maps = []
    for c in range(NCORES):
        b, ks = divmod(c, NCORES // B)
        sl = slice(KSH * ks, KSH * (ks + 1))
        # efT[p, j] = expfac_shard[(128j + p) mod 1024]
        ef = np.asarray(expfac[sl], dtype=np.float32).reshape(8, 128).T  # [128, 8]
        efT = np.concatenate([ef, ef], axis=1)                           # [128, 16]
        in_maps.append({
            "coordsT": np.ascontiguousarray(coords[b].T, dtype=np.float32),
            "qT": np.ascontiguousarray(q[b].reshape(NCH, 128).T, dtype=np.float32),
            "kmodT": np.ascontiguousarray(
                cell_inv.astype(np.float32).T @ kvec[sl].T.astype(np.float32)),
            "efT": np.ascontiguousarray(efT),
        })
    return in_maps


def _finalize(results, q, volume, bewald):
    recip = np.zeros((B, N), np.float32)
    for c in range(NCORES):
        b = c // (NCORES // B)
        recip[b] += results[c]["recip"].T.reshape(-1)
    scale1 = np.float32(BOHR / (math.pi * float(volume[0])))
    scale2 = np.float32(2.0 * float(bewald[0]) * BOHR / math.sqrt(math.pi))
    phi = (recip * scale1 - q.astype(np.float32) * scale2).astype(np.float32)
    e = (np.float32(0.5) * q.astype(np.float32) * phi).astype(np.float32)
    return e, phi


def kernel(coords, q, cell_inv, kvec, expfac, volume, bewald):
    from concourse.bass_utils import run_bass_kernel_spmd

    nc = _get_prog()
    in_maps = _make_in_maps(coords, q, cell_inv, kvec, expfac)
    res = run_bass_kernel_spmd(nc, in_maps, list(range(NCORES))).results
    return _finalize(res, q, volume, bewald)


# revision 7
# speedup vs baseline: 1.5396x; 1.0032x over previous
"""Ewald reciprocal-space kernel for Trainium2 (8 NeuronCores, SPMD) — v3.

Math (per batch b):
    s        = cell_inv @ x          (fractional coords)
    theta    = 2*pi * (kvec . s)     (B, N, NK) phases
    S_re/S_im= sum_n q_n {cos,sin}(theta)          (structure factor)
    recip_n  = sum_k expfac_k (S_re cos + S_im sin)
    phi      = recip * BOHR/(pi*V) - q * 2*bewald*BOHR/sqrt(pi)
    returns (0.5*q*phi, phi)

Sharding: 8 cores = 2 batches x 4 k-shards (1024 k-vectors each). Each core
computes its full-N, shard-K contribution to recip with no collectives; host
sums the 4 shard partials per batch and applies the final affine.

Device pipeline per core (N=4096 as 32 chunks of 128 partitions):
  u  = x . kmod (kmod = Cinv^T k)       fp32r matmul into PSUM
  rn = (u + M) - M                      magic round; DVE or ACT-assisted
  f  = rn - u  in [-1/2, 1/2]           DVE scalar_tensor_tensor, fp16 out
  g  = |f|     (uint16 AND 0x7FFF)      DVE 4x-mode fp16
  h  = g - 1/4                          GPSIMD (pool), fp16
  sin(theta) = Sin(-2pi f), cos(theta) = Sin(-2pi h)   one ACT Sin per 2 chunks
  S rows: 4 PE matmuls/chunk, out [1, 512] parked at quadrant partition 32q
     (4 concurrent PSUM groups share one 2KB region on disjoint partitions)
  cs chunks DMA-transposed (xbar) into csT[k-slice partitions, n free]
  S extraction: 4 single-partition row copies (DVE+ACT), then 16 tiny PE
     transposes [1,128]->[128,1] into one PSUM group; w = efT * S^T (DVE)
  recip: free-size-1 weight-stationary PE matmuls (lhsT = csT block,
     rhs = w column), accumulated over 16 k-slices; PSUM ring reuse with
     per-pair copy-out, recip emitted as [128, 32]
"""

import math
from contextlib import ExitStack

import numpy as np

BOHR = 1.8897261258369282

B, N, NK = 2, 4096, 4096
NCORES = 8
KSH = NK // 4          # k-vectors per core
NCH = N // 128         # 32 n-chunks
CW = 2 * KSH           # cs chunk width: [sin | cos]
NSL = CW // 128        # 16 k-slices per chunk
MAGIC = 12582912.0     # 1.5 * 2**23: fp32 round-to-nearest-integer
NEG2PI = -6.28318452835083  # two fp32 ulps below 2*pi

_PROG = {}


def _build_program(reps: int = 1, stage: str = 'full', n_assist: int = 14,
                   mm_bufs: int = 3, rn_bufs: int = 2, cs_bufs: int = 2,
                   extr_act: int = 4, p2_act: int = 1, assist_last: int = NCH,
                   assist0: bool = False,
                   split_sins: tuple = (0, 1, NCH // 2 - 2, NCH // 2 - 1)):
    import concourse.bass as bass
    import concourse.bacc as bacc
    import concourse.tile as tile
    import concourse.mybir as mybir

    F32 = mybir.dt.float32
    F32R = mybir.dt.float32r
    F16 = mybir.dt.float16
    U16 = mybir.dt.uint16
    ADD = mybir.AluOpType.add
    SUB = mybir.AluOpType.subtract
    MULT = mybir.AluOpType.mult
    AND = mybir.AluOpType.bitwise_and
    SIN = mybir.ActivationFunctionType.Sin
    COPY = mybir.ActivationFunctionType.Copy

    nc = bacc.Bacc(trn_type="TRN2", target_bir_lowering=False, debug=False)

    coordsT_d = nc.dram_tensor("coordsT", [3, N], F32, kind="ExternalInput").ap()
    qT_d = nc.dram_tensor("qT", [128, NCH], F32, kind="ExternalInput").ap()
    kmodT_d = nc.dram_tensor("kmodT", [3, KSH], F32, kind="ExternalInput").ap()
    efT_d = nc.dram_tensor("efT", [128, NSL], F32, kind="ExternalInput").ap()
    recip_d = nc.dram_tensor("recip", [128, NCH], F32, kind="ExternalOutput").ap()

    # chunks whose rounding runs on the scalar engine (DVE/ACT balance);
    # assist_last bounds the last assisted chunk so the ACT tail is Sin-only
    ASSIST = {int((i + 0.5) * assist_last / n_assist) for i in range(n_assist)}
    if assist0:
        ASSIST.add(0)

    with tile.TileContext(nc) as tc, ExitStack() as ctx:
        const = ctx.enter_context(tc.tile_pool(name="const", bufs=1))
        pu = ctx.enter_context(tc.tile_pool(name="pu", bufs=3, space="PSUM"))
        pacc = ctx.enter_context(tc.tile_pool(name="pacc", bufs=1, space="PSUM"))
        wk_rn = ctx.enter_context(tc.tile_pool(name="wk_rn", bufs=rn_bufs))
        wk_mm = ctx.enter_context(tc.tile_pool(name="wk_mm", bufs=mm_bufs))
        wk_cs = ctx.enter_context(tc.tile_pool(name="wk_cs", bufs=cs_bufs))
        wk_out = ctx.enter_context(tc.tile_pool(name="wk_out", bufs=1))

        # ---- load inputs (first chunk's dependencies in tiny pieces first) ----
        cts = const.tile([3, N], F32R)
        kmod = const.tile([3, KSH], F32R, name="kmod")
        nc.sync.dma_start(out=kmod[:, 0:512], in_=kmodT_d[:, 0:512].bitcast(F32R))
        nc.scalar.dma_start(out=cts[:, 0:384], in_=coordsT_d[:, 0:384].bitcast(F32R))
        nc.sync.dma_start(out=kmod[:, 512:1024],
                          in_=kmodT_d[:, 512:1024].bitcast(F32R))
        nc.sync.dma_start(out=cts[:, 384:1024],
                          in_=coordsT_d[:, 384:1024].bitcast(F32R))
        for hh in range(1024, N, 1024):
            nc.sync.dma_start(
                out=cts[:, hh:hh + 1024],
                in_=coordsT_d[:, hh:hh + 1024].bitcast(F32R),
            )
        qt = const.tile([128, NCH], F32)
        nc.sync.dma_start(out=qt[:, :], in_=qT_d)
        efT = const.tile([128, NSL], F32)
        nc.sync.dma_start(out=efT[:, :], in_=efT_d)
        qt16 = const.tile([128, NCH], F16)
        one1 = const.tile([1, 1], F32)

        # persistent: the transposed trig store and the S quadrant rows
        csT = const.tile([128, NSL, N], F16)   # [k-in-slice][slice j][n]
        sgrid = pacc.tile([128, 512], F32, tag="sg")

        def emit_u(t):
            u_ps = pu.tile([128, KSH], F32, tag="u", name=f"u{t}")
            for hh in range(0, KSH, 512):
                nc.tensor.matmul(
                    u_ps[:, hh:hh + 512],
                    lhsT=cts[:, 128 * t:128 * (t + 1)],
                    rhs=kmod[:, hh:hh + 512],
                    start=True, stop=True,
                )
            return u_ps

        us = {t: emit_u(t) for t in range(3)}
        for s in range(NCH // 2):
            mmp = wk_mm.tile([128, 2 * CW], F16, tag="mm", name=f"mm{s}")
            for c in (0, 1):
                t = 2 * s + c
                u_ps = us.pop(t)
                fsl = mmp[:, CW * c:CW * c + KSH]            # f (sin half)
                hsl = mmp[:, CW * c + KSH:CW * (c + 1)]      # h (cos half)
                rn = wk_rn.tile([128, KSH], F32, tag="rn", name=f"rn{t}")
                if t in ASSIST and stage != 'rr0':
                    # ACT: v = u + M (rounds); DVE stt: (v - M) - u
                    nc.scalar.activation(rn[:, :], u_ps[:, :], COPY,
                                         bias=MAGIC, scale=1.0)
                    s0 = MAGIC
                else:
                    nc.vector.tensor_scalar(
                        out=rn[:, :], in0=u_ps[:, :],
                        scalar1=MAGIC, scalar2=MAGIC, op0=ADD, op1=SUB,
                    )
                    s0 = 0.0
                nc.vector.scalar_tensor_tensor(
                    out=fsl, in0=rn[:, :], scalar=s0, in1=u_ps[:, :],
                    op0=ADD if s0 == 0.0 else SUB, op1=SUB,
                )
                # g = |f| via uint16 AND; h = g - 1/4 on the pool engine
                nc.vector.tensor_scalar(
                    out=hsl.bitcast(U16), in0=fsl.bitcast(U16),
                    scalar1=0x7FFF, scalar2=None, op0=AND,
                )
                nc.gpsimd.tensor_scalar(
                    out=hsl, in0=hsl, scalar1=0.25, scalar2=None, op0=SUB,
                )
                tn = t + 3
                if tn < NCH:
                    us[tn] = emit_u(tn)
            if s == 0:
                nc.vector.tensor_copy(qt16[:, :], qt[:, :])
                nc.vector.memset(one1[:, :], 1.0)
            if stage == 'rr':
                continue
            # sin(theta) = Sin(-2pi f); cos(theta) = Sin(-2pi h)
            csp = wk_cs.tile([128, 2 * CW], F16, tag="cs", name=f"cs{s}")
            if s in split_sins:
                nc.scalar.activation(csp[:, 0:CW], mmp[:, 0:CW], SIN,
                                     bias=0.0, scale=NEG2PI)
            else:
                nc.scalar.activation(csp[:, :], mmp[:, :], SIN,
                                     bias=0.0, scale=NEG2PI)
            for c in (0, 1):
                t = 2 * s + c
                if s in split_sins and c == 1:
                    nc.scalar.activation(csp[:, CW:], mmp[:, CW:], SIN,
                                         bias=0.0, scale=NEG2PI)
                for qd in range(4):
                    nc.tensor.matmul(
                        sgrid[32 * qd:32 * qd + 1, 0:512],
                        lhsT=qt16[:, t:t + 1],
                        rhs=csp[:, CW * c + 512 * qd:CW * c + 512 * (qd + 1)],
                        start=(t == 0), stop=(t == NCH - 1),
                        tile_position=(0, 32 * qd),
                    )
                if stage != 'act':
                    # csT[p, j, 128t + n] = csp[n, CW*c + 128j + p]
                    nc.sync.dma_start_transpose(
                        out=csT[:, :, 128 * t:128 * (t + 1)],
                        in_=csp[:, CW * c:CW * (c + 1)],
                    )

        if stage != 'full':
            zz = wk_out.tile([128, NCH], F32, name="zz")
            nc.vector.memset(zz[:, :], 0.0)
            nc.sync.dma_start(out=recip_d, in_=zz[:, :])
        else:
            # ---- S extraction: 4 quadrant rows -> SBUF (DVE + ACT) ----
            sgq0 = const.tile([1, 512], F32, name="sgq0")
            sgq1 = const.tile([1, 512], F32, name="sgq1")
            sgq2 = const.tile([1, 512], F32, name="sgq2")
            sgq3 = const.tile([1, 512], F32, name="sgq3")
            sgq = [sgq0, sgq1, sgq2, sgq3]
            wtp = pacc.tile([128, 512], F32, tag="wtp")
            for qd in range(4):
                if qd >= extr_act:
                    nc.vector.tensor_copy(sgq[qd][:, :],
                                          sgrid[32 * qd:32 * qd + 1, 0:512])
                else:
                    nc.scalar.copy(sgq[qd][:, :],
                                   sgrid[32 * qd:32 * qd + 1, 0:512])
                for aa in range(4):
                    j = 4 * qd + aa
                    nc.tensor.matmul(
                        wtp[:, j:j + 1],
                        lhsT=sgq[qd][0:1, 128 * aa:128 * (aa + 1)],
                        rhs=one1[:, :],
                        is_transpose=True, start=(j == 0), stop=(j == NSL - 1),
                    )
            wcolT = const.tile([128, NSL], F16, tag="wcolT")
            nc.vector.tensor_tensor(
                out=wcolT[:, :], in0=wtp[:, 0:NSL], in1=efT[:, :], op=MULT,
            )

            # ---- pass 2: recip[n] = sum_j csT[:, j, n]^T wcolT[:, j] ----
            rr = wk_out.tile([128, NCH], F32, name="rr")
            for m in range(NCH // 2):
                rp = pu.tile([128, KSH], F32, tag="u", name=f"rp{m}")
                for c in (0, 1):
                    t = 2 * m + c
                    for j in range(NSL):
                        nc.tensor.matmul(
                            rp[:, 512 * c:512 * c + 1],
                            lhsT=csT[:, j, 128 * t:128 * (t + 1)],
                            rhs=wcolT[:, j:j + 1],
                            start=(j == 0), stop=(j == NSL - 1),
                        )
                if m % 4 < p2_act:
                    nc.scalar.copy(rr[:, 2 * m:2 * m + 2], rp[:, 0:KSH:512])
                else:
                    nc.vector.tensor_copy(rr[:, 2 * m:2 * m + 2],
                                          rp[:, 0:KSH:512])
            nc.sync.dma_start(out=recip_d, in_=rr[:, :])

    nc.compile()
    return nc


def _get_prog(reps: int = 1, stage: str = "full", **kw):
    key = (reps, stage, tuple(sorted(kw.items())))
    if key not in _PROG:
        _PROG[key] = _build_program(reps, stage, **kw)
    return _PROG[key]


def _make_in_maps(coords, q, cell_inv, kvec, expfac):
    in_maps = []
    for c in range(NCORES):
        b, ks = divmod(c, NCORES // B)
        sl = slice(KSH * ks, KSH * (ks + 1))
        # efT[p, j] = expfac_shard[(128j + p) mod 1024]
        ef = np.asarray(expfac[sl], dtype=np.float32).reshape(8, 128).T  # [128, 8]
        efT = np.concatenate([ef, ef], axis=1)                           # [128, 16]
        in_maps.append({
            "coordsT": np.ascontiguousarray(coords[b].T, dtype=np.float32),
            "qT": np.ascontiguousarray(q[b].reshape(NCH, 128).T, dtype=np.float32),
            "kmodT": np.ascontiguousarray(
                cell_inv.astype(np.float32).T @ kvec[sl].T.astype(np.float32)),
            "efT": np.ascontiguousarray(efT),
        })
    return in_maps


def _finalize(results, q, volume, bewald):
    recip = np.zeros((B, N), np.float32)
    for c in range(NCORES):
        b = c // (NCORES // B)
        recip[b] += results[c]["recip"].T.reshape(-1)
    scale1 = np.float32(BOHR / (math.pi * float(volume[0])))
    scale2 = np.float32(2.0 * float(bewald[0]) * BOHR / math.sqrt(math.pi))
    phi = (recip * scale1 - q.astype(np.float32) * scale2).astype(np.float32)
    e = (np.float32(0.5) * q.astype(np.float32) * phi).astype(np.float32)
    return e, phi


def kernel(coords, q, cell_inv, kvec, expfac, volume, bewald):
    from concourse.bass_utils import run_bass_kernel_spmd

    nc = _get_prog()
    in_maps = _make_in_maps(coords, q, cell_inv, kvec, expfac)
    res = run_bass_kernel_spmd(nc, in_maps, list(range(NCORES))).results
    return _finalize(res, q, volume, bewald)
